# revision 25
# baseline (speedup 1.0000x reference)
"""Multi-head attention (B=4, N=2048, C=1024, H=16, D=64) on 8 trn2 cores.

Sharding: core c handles batch b = c//2 and head-group g = c%2 (8 heads,
512 channels). No collectives: the row-parallel output projection partials
are summed on host (2 cores per batch), with bp + bv@Wp folded in on host
(softmax rows sum to 1, so the v-bias contributes a constant per channel).

Device layout is transposed end-to-end (tokens on the free axis):
  xT [C, N] -> Q^T/K^T pair tiles [128, N] (rows 0:64 head 2p, 64:128 head
  2p+1) -> S^T tiles [keys, queries] via K=64 matmuls -> exp on ACT (no max
  subtraction; scores are O(1) by construction) -> exp output is fp8e4 and
  PV runs in fp8 DoubleRow over key-tile PAIRS (contraction 256): V_aug
  per pair tile [vA | ones64 | vB] x {k=0,1} interleaved; psum rows split
  into O_u and a 64-row replicated rowsum -> DVE reciprocal * mul -> O^T ->
  Y^T = Wp_g^T O^T (bf16).

Schedule: the kernel is ACT(exp)-bound (256 exps x ~1.07us = 274us); the
whole schedule aims to keep ACT 100% busy from ~10us on. Prologue is only
wq+x0+wk DMA and the pair-0 chunk-0 Q/K projection; all other projections
(V for all 16 key tiles, remaining Q/K chunks, later pairs, output
projection) are deadline-scheduled as PE filler inside the attention
strips. A dummy-matmul burst at t0 (on the resident ones tile) keeps the
PE HAM clock gate warm through the initial DMA wait so the first real
matmuls run at 2.4 GHz.

Matmul operand dtypes: bf16 for projections and S (accuracy-critical),
fp8e4m3 for the PV DoubleRow pair (E = exp output, V_aug); accumulation
is always fp32 in PSUM. Measured fro rel err ~1.4e-2 (gate 2e-2).
"""

import os
import sys

sys.path.insert(0, "/opt/trn_rl_repo")

import numpy as np

B, N, C, H = 4, 2048, 1024, 16
D = C // H
SCALE = D**-0.5
NCORES = 8
FC = 512  # channels per core
NP = 4  # head pairs per core
KT8 = C // 128  # contraction tiles
NCQ = N // 512  # n-chunks of 512
NMT = N // 128  # key tiles
NKP = NMT // 2  # key-tile pairs (DoubleRow)

_nc = None


def _cap(ap_slice, block_step, nblocks, width):
    """2-free-dim AP: nblocks blocks of `width` cols, stride block_step."""
    import concourse.bass as bass

    lst = [list(p) for p in ap_slice.ap]
    assert len(lst) == 2 and lst[1][0] == 1, lst
    return bass.AP(
        ap_slice.tensor, ap_slice.offset, [lst[0], [block_step, nblocks], [1, width]]
    )


def _ap3(ap_slice, d1_step, d1_n, d2_step, d2_n):
    """3-free-dim AP for DoubleRow operands: [part, [d1_step,d1_n], [d2_step,d2_n]]."""
    import concourse.bass as bass

    lst = [list(p) for p in ap_slice.ap]
    return bass.AP(
        ap_slice.tensor, ap_slice.offset, [lst[0], [d1_step, d1_n], [d2_step, d2_n]]
    )


def _build():
    import concourse.bacc as bacc
    import concourse.mybir as mybir
    import concourse.tile as tile

    F32 = mybir.dt.float32
    BF16 = mybir.dt.bfloat16
    F8 = mybir.dt.float8e4
    AF = mybir.ActivationFunctionType
    DR = mybir.MatmulPerfMode.DoubleRow

    nc = bacc.Bacc("TRN2", target_bir_lowering=False, debug=False, num_devices=NCORES)

    xT_d = nc.dram_tensor("xT", (NCQ, 128, KT8 * 512), BF16, kind="ExternalInput").ap()
    # wq/wk are pair-major [128, NP, KT8, 128] so pair 0's slices can DMA
    # first; wv keeps the k-major merged layout (its matmuls read full rows).
    wq_d = nc.dram_tensor("wq", (128, KT8 * FC), BF16, kind="ExternalInput").ap()
    wk_d = nc.dram_tensor("wk", (128, KT8 * FC), BF16, kind="ExternalInput").ap()
    wv_d = nc.dram_tensor("wv", (128, KT8 * FC), BF16, kind="ExternalInput").ap()
    wp_d = nc.dram_tensor("wp", (128, NP * C), BF16, kind="ExternalInput").ap()
    bq_d = nc.dram_tensor("bq", (128, NP), F32, kind="ExternalInput").ap()
    bk_d = nc.dram_tensor("bk", (128, NP), F32, kind="ExternalInput").ap()
    on_d = nc.dram_tensor("ones", (128, 512), F8, kind="ExternalInput").ap()
    yT_d = nc.dram_tensor("yT", (C // 128, NCQ, 128, 512), BF16, kind="ExternalOutput").ap()

    with tile.TileContext(nc) as tc:
        with (
            tc.tile_pool(name="sb", bufs=1) as sb,
            tc.tile_pool(name="pe_", bufs=3) as pbe,
            tc.tile_pool(name="prc", bufs=4) as prc,
            tc.tile_pool(name="pyb", bufs=4) as pyb,
            tc.tile_pool(name="psqk", bufs=2, space="PSUM") as psqk,
            tc.tile_pool(name="psa", bufs=2, space="PSUM") as psa,
            tc.tile_pool(name="pso", bufs=2, space="PSUM") as pso,
        ):
            # ---- resident tiles + DMAs ----
            QT = [sb.tile([128, N], BF16, name=f"qt{p}") for p in range(NP)]
            KT = [sb.tile([128, N], BF16, name=f"kt{p}") for p in range(NP)]
            # V_aug DoubleRow tiles: per key-tile pair t, layout
            # [128 keys, (2p+k)*192 + {vA 0:64 | ones 64:128 | vB 128:192}]
            VA = [sb.tile([128, 192 * NP * 2], F8, name=f"va{t}") for t in range(NKP)]
            OT = [sb.tile([128, N], BF16, name=f"ot{p}") for p in range(NP)]
            bq_t = sb.tile([128, NP], F32, name="bq_t")
            bk_t = sb.tile([128, NP], F32, name="bk_t")
            on_t = sb.tile([128, 512], F8, name="on_t")
            z_t = sb.tile([128, 128], BF16, name="z_t")
            nc.sync.dma_start(out=bq_t[:], in_=bq_d)
            nc.sync.dma_start(out=bk_t[:], in_=bk_d)
            nc.sync.dma_start(out=on_t[:], in_=on_d)
            nc.vector.memset(z_t[:], 0.0)

            # ---- HAM warm-up at t0: dummy matmuls on the ones tile keep the
            # PE activity monitor busy through the wq/x0 DMA wait so the
            # first projection matmuls run at 2.4 GHz.
            warm = psqk.tile([128, 512], F32, tag="qk", name="warm")
            for _ in range(16):
                nc.tensor.matmul(
                    warm[:], on_t[:, 0:128], on_t[:],
                    start=True, stop=True, skip_group_check=True,
                )

            # DMA priority: wq/wk pair-0 slices + xT chunk 0 first (gates the
            # first exp), wv next (V filler for the first strip), then the
            # remaining pairs' wq/wk, remaining xT, wp last.
            PW = KT8 * 128  # per-pair width in the pair-major wq/wk layout
            wq_all = sb.tile([128, KT8 * FC], BF16, name="wq_all")
            wk_all = sb.tile([128, KT8 * FC], BF16, name="wk_all")
            wv_all = sb.tile([128, KT8 * FC], BF16, name="wv_all")
            # Inputs are split across BOTH HWDGE rings (sync/SP and scalar/
            # ACT sequencers): a single ring streams descriptors FIFO and
            # sustains only ~70 GB/s here, which starved the first strip.
            # Each x chunk is also split in column halves, one per ring, so
            # low-k matmuls can start before the full chunk lands.
            HW2 = KT8 * 256
            nc.sync.dma_start(out=wq_all[:, 0:PW], in_=wq_d[:, 0:PW])
            nc.scalar.dma_start(out=wk_all[:, 0:PW], in_=wk_d[:, 0:PW])

            def _xall(ncq):
                t = sb.tile([128, KT8 * 512], BF16, name=f"xt_{ncq}")
                nc.sync.dma_start(out=t[:, 0:HW2], in_=xT_d[ncq, :, 0:HW2])
                nc.scalar.dma_start(out=t[:, HW2:], in_=xT_d[ncq, :, HW2:])
                return t

            xt_all = [_xall(0), _xall(1)]
            nc.sync.dma_start(out=wv_all[:, 0:HW2], in_=wv_d[:, 0:HW2])
            nc.scalar.dma_start(out=wv_all[:, HW2:], in_=wv_d[:, HW2:])
            xt_all.append(_xall(2))
            xt_all.append(_xall(3))
            # Later pairs' weights and wp are DMA'd from inside the step
            # loop (the HWDGE queues interleave all pending transfers, so
            # issuing everything at t0 starves the chunks the first strip
            # is already waiting on). A dummy DVE read of the destination
            # creates a WAR hazard that holds the trigger back to the wall
            # time of the step it is emitted at.
            wp_all = sb.tile([128, NP * C], BF16, name="wp_all")
            gate_t = sb.tile([128, 1], BF16, name="gate_t")

            def emit_w_dma(p):
                def go():
                    nc.vector.tensor_copy(gate_t[:], wq_all[:, PW * p : PW * p + 1])
                    nc.sync.dma_start(out=wq_all[:, PW * p : PW * (p + 1)],
                                      in_=wq_d[:, PW * p : PW * (p + 1)])
                    nc.vector.tensor_copy(gate_t[:], wk_all[:, PW * p : PW * p + 1])
                    nc.sync.dma_start(out=wk_all[:, PW * p : PW * (p + 1)],
                                      in_=wk_d[:, PW * p : PW * (p + 1)])
                return go

            def emit_wp_dma():
                nc.vector.tensor_copy(gate_t[:], wp_all[:, 0:1])
                nc.sync.dma_start(out=wp_all[:], in_=wp_d)

            # ones-fill of the VA DoubleRow tiles (DVE, doesn't touch PE):
            # blocks (2p+k)*192 + 64:128 for all 8 (p,k) -> stride 192 x 8.
            for t in range(NKP):
                nc.vector.tensor_copy(
                    _ap3(VA[t][:, 64:65], 192, NP * 2, 1, 64), on_t[:]
                )

            # ---- QKV emission helpers ----
            def emit_qk_group(p, proj, ncq):
                """One 8-matmul psum group (+ DVE bias evac) for pair p.
                Returns list of closures emitting one instruction each."""
                w_all, bias_t, dst = (
                    (wq_all, bq_t, QT) if proj == 0 else (wk_all, bk_t, KT)
                )
                cs = slice(512 * ncq, 512 * (ncq + 1))
                state = {}

                def mk_mm(k):
                    def go():
                        if "pq" not in state:
                            state["pq"] = psqk.tile(
                                [128, 512], F32, tag="qk", name=f"pq_{p}_{proj}_{ncq}"
                            )
                        nc.tensor.matmul(
                            state["pq"][:],
                            w_all[:, PW * p + 128 * k : PW * p + 128 * (k + 1)],
                            xt_all[ncq][:, 512 * k : 512 * (k + 1)],
                            start=(k == 0), stop=(k == KT8 - 1), skip_group_check=True,
                        )

                    return go

                def evac():
                    nc.vector.tensor_scalar_add(
                        dst[p][:, cs], state["pq"][:], bias_t[:, p : p + 1]
                    )

                return [mk_mm(k) for k in range(KT8)] + [evac]

            def emit_v_group(nt):
                """V projection for key tile nt -> VA[nt//2] slot k=nt%2."""
                ncq, tt = divmod(nt, 4)
                t, k = divmod(nt, 2)
                state = {}

                def mk_mm(kk):
                    def go():
                        if "pv" not in state:
                            state["pv"] = psqk.tile(
                                [128, 512], F32, tag="qk", name=f"pv_{nt}"
                            )
                        nc.tensor.matmul(
                            state["pv"][:],
                            xt_all[ncq][:, 512 * kk + 128 * tt : 512 * kk + 128 * (tt + 1)],
                            wv_all[:, FC * kk : FC * (kk + 1)],
                            start=(kk == 0), stop=(kk == KT8 - 1), skip_group_check=True,
                        )

                    return go

                def evac():
                    va = VA[t]
                    pv = state["pv"]
                    base = 192 * k
                    nc.vector.tensor_copy(
                        _ap3(va[:, base : base + 1], 384, NP, 1, 64),
                        _cap(pv[:, 0:64], 128, NP, 64),
                    )
                    nc.vector.tensor_copy(
                        _ap3(va[:, base + 128 : base + 129], 384, NP, 1, 64),
                        _cap(pv[:, 64:128], 128, NP, 64),
                    )

                return [mk_mm(kk) for kk in range(KT8)] + [evac]

            # ---- output projection chunk emitter ----
            def emit_proj_group(c, ncq):
                """Y^T chunk: 4 accumulating matmuls + DVE evac + DMA out."""
                cs = slice(512 * ncq, 512 * (ncq + 1))
                state = {}

                def mk_mm(f):
                    def go():
                        if "py" not in state:
                            state["py"] = psqk.tile(
                                [128, 512], F32, tag="qk", name=f"py_{c}_{ncq}"
                            )
                        nc.tensor.matmul(
                            state["py"][:],
                            wp_all[:, C * f + 128 * c : C * f + 128 * (c + 1)],
                            OT[f][:, cs],
                            start=(f == 0), stop=(f == NP - 1), skip_group_check=True,
                        )

                    return go

                def evac():
                    yb = pyb.tile([128, 512], BF16, tag="yb", name=f"yb_{c}_{ncq}")
                    nc.vector.tensor_copy(yb[:], state["py"][:])
                    nc.sync.dma_start(out=yT_d[c, ncq, :, :], in_=yb[:])

                return [mk_mm(f) for f in range(NP)] + [evac]

            # ---- deadline-scheduled filler groups ----
            # Each group's closures are spread EVENLY across steps
            # [avail, deadline] at build time. Even spreading (not
            # earliest-first) matters: draining filler early leaves the PE
            # micro-idling in later strips, which trips the HAM clock gate
            # back to 1.2 GHz and the whole pipeline slows ~1.5x.
            NSTEP = NP * NCQ * NMT
            emit_at = [[] for _ in range(NSTEP)]
            drain = []

            def add_group(avail, deadline, closures):
                closures = list(closures)
                if avail >= NSTEP:
                    drain.extend(closures)
                    return
                a = max(avail, 0)
                d = min(max(deadline, a), NSTEP - 1)
                span = d - a
                for j, c in enumerate(closures):
                    s = a + min(span, (j * (span + 1)) // len(closures))
                    emit_at[s].append(c)

            # V projections: VA pair t needed by PV at step 2t+1. avail is
            # aligned with the xT chunk DMA arrival so a stalled V matmul
            # doesn't head-of-line-block the in-order PE queue.
            for nt in range(NMT):
                av = 0 if nt < 4 else 4 * (nt // 4) - 1
                add_group(av, max(nt, 1), emit_v_group(nt))
            # pair-0 remaining Q/K chunks (chunk 0 is the prologue):
            for cq in range(1, NCQ):
                add_group(4 * cq - 3, 4 * cq - 2, emit_qk_group(0, 1, cq))  # K
                add_group(4 * cq - 1, 16 * cq - 2, emit_qk_group(0, 0, cq))  # Q
            # later pairs' Q/K (deadline 64p-2: the S at step 64p is emitted
            # during step 64p-1, so operands must be fully emitted before);
            # their weight DMAs are triggered from inside the loop first:
            add_group(14, 14, [emit_w_dma(1)])
            add_group(38, 38, [emit_w_dma(2), emit_w_dma(3)])
            add_group(60, 60, [emit_wp_dma])
            for p in range(1, NP):
                base = 64 * p
                av = max(18, base - 56)
                for cq in range(NCQ):
                    add_group(av, base + 4 * cq - 2, emit_qk_group(p, 1, cq))
                    add_group(av, base + 16 * cq - 2, emit_qk_group(p, 0, cq))
            # output projection for qc: available after strip (p3, qc) ends;
            # staggered avail so the 8 c-groups don't all land on the same
            # steps.
            for cq in range(NCQ):
                av = 192 + 16 * cq + 16
                for c in range(C // 128):
                    add_group(av + c, av + 15, emit_proj_group(c, cq))

            # ---- prologue: pair-0 chunk-0 Q/K only ----
            for proj in range(2):
                for go in emit_qk_group(0, proj, 0):
                    go()

            # ---- attention strips ----
            # strip = (pair, 512-query-chunk). Both heads of the pair share
            # one [128, 1024] S psum tile: head A (rows 0:64 of K^T/Q^T) ->
            # cols 0:512, head B (rows 64:128) -> cols 512:1024; the two K=64
            # matmuls row-pack onto disjoint PE row-groups and run
            # concurrently. One exp covers both heads and writes fp8 into
            # half of the current E pair tile; PV fires every second step as
            # two DoubleRow matmuls over the key-tile pair (contraction 256).
            # Software-pipelined: S(i+1) is emitted before PV(i//2).
            steps = [(p, qc, mt) for p in range(NP) for qc in range(NCQ) for mt in range(NMT)]

            def emit_S(p, qc, mt):
                qs = slice(512 * qc, 512 * (qc + 1))
                ms = slice(128 * mt, 128 * (mt + 1))
                sa = psa.tile([128, 1024], F32, tag="sa", name=f"sa_{p}_{qc}_{mt}")
                nc.tensor.matmul(
                    sa[:, 0:512], KT[p][0:64, ms], QT[p][0:64, qs],
                    start=True, stop=True,
                )
                nc.tensor.matmul(
                    sa[:, 512:1024], KT[p][64:128, ms], QT[p][64:128, qs],
                    start=True, stop=True,
                )
                return sa

            ots = None
            et = None
            sa_next = emit_S(*steps[0])
            for i, (p, qc, mt) in enumerate(steps):
                if mt == 0:
                    ots = [
                        pso.tile([128, 512], F32, tag="o", name=f"o_{p}_{qc}_{j}")
                        for j in range(2)
                    ]
                if mt % 2 == 0:
                    et = pbe.tile([128, 2048], F8, tag="e", name=f"e_{p}_{qc}_{mt}")
                sa_cur = sa_next
                nc.scalar.activation(
                    et[:, 1024 * (mt % 2) : 1024 * (mt % 2) + 1024],
                    sa_cur[:], AF.Exp, scale=SCALE,
                )
                if i + 1 < len(steps):
                    sa_next = emit_S(*steps[i + 1])
                for go in emit_at[i]:
                    go()
                # pad the PE with zero-weight matmuls accumulating +0 into the
                # live O tile whenever filler is scarce: micro-idle would trip
                # the HAM clock gate back to 1.2 GHz. Skipped at mt<2: there
                # the O tile is freshly pool-rotated and a write would stall
                # the PE behind the previous strip's DVE writeback.
                if mt >= 2:
                    for _ in range(max(0, 2 - len(emit_at[i]))):
                        nc.tensor.matmul(
                            ots[1], z_t[:], QT[p][:, 0:512],
                            start=False, stop=False, skip_group_check=True,
                        )
                if mt % 2 == 1:
                    t = mt // 2
                    first, last = t == 0, t == NKP - 1
                    va = VA[t]
                    nc.tensor.matmul(
                        ots[0],
                        _ap3(va[:, 384 * p : 384 * p + 1], 192, 2, 1, 128),
                        _ap3(et[:, 0:1], 1024, 2, 1, 512),
                        start=first, stop=last, skip_group_check=True,
                        perf_mode=DR,
                    )
                    nc.tensor.matmul(
                        ots[1],
                        _ap3(va[:, 384 * p + 64 : 384 * p + 65], 192, 2, 1, 128),
                        _ap3(et[:, 512:513], 1024, 2, 1, 512),
                        start=first, stop=last, skip_group_check=True,
                        perf_mode=DR,
                    )
                if mt == NMT - 1:
                    qs = slice(512 * qc, 512 * (qc + 1))
                    for j in range(2):
                        o = ots[j]
                        # reciprocal_approx_fast mis-executes at base partition
                        # != 0: run it over the whole tile (unused rows produce
                        # garbage that is never read) and slice after.
                        rc = prc.tile([128, 512], F32, tag="rc", name=f"rc_{p}_{qc}_{j}")
                        nc.vector.reciprocal_approx_fast(rc[:], o[:])
                        osl, rcl = (
                            (o[0:64, :], rc[64:128, :]) if j == 0 else (o[64:128, :], rc[0:64, :])
                        )
                        nc.vector.tensor_mul(OT[p][64 * j : 64 * j + 64, qs], osl, rcl)
            # drain remaining fillers (final output projection chunks)
            for go in drain:
                go()

    nc.compile()
    return nc


def _get_nc():
    global _nc
    if _nc is None:
        try:
            import jax

            jax.config.update(
                "jax_compilation_cache_dir", os.path.expanduser("~/.cache/jax_bass")
            )
            jax.config.update("jax_persistent_cache_min_compile_time_secs", 0.0)
            jax.config.update("jax_persistent_cache_min_entry_size_bytes", 0)
        except Exception:
            pass
        _nc = _build()
    return _nc


def _wmerge(w, mdt):
    """(KT*128, F) -> [128, KT*F] partition-major merged layout."""
    kt = w.shape[0] // 128
    return np.ascontiguousarray(
        w.reshape(kt, 128, w.shape[1]).transpose(1, 0, 2).reshape(128, kt * w.shape[1]).astype(mdt)
    )


def _wmerge_pm(w, mdt):
    """(KT*128, NP*128) -> [128, NP*KT*128] pair-major merged layout:
    out[r, (p*KT + k)*128 + m] = w[128k + r, 128p + m]."""
    kt = w.shape[0] // 128
    npairs = w.shape[1] // 128
    return np.ascontiguousarray(
        w.reshape(kt, 128, npairs, 128)
        .transpose(1, 2, 0, 3)
        .reshape(128, npairs * kt * 128)
        .astype(mdt)
    )


def make_in_maps(inputs):
    import ml_dtypes

    mdt = ml_dtypes.bfloat16
    f8 = ml_dtypes.float8_e4m3fn
    x = np.asarray(inputs["x"], np.float32)
    Wq = np.asarray(inputs["Wq"], np.float32)
    Wk = np.asarray(inputs["Wk"], np.float32)
    Wv = np.asarray(inputs["Wv"], np.float32)
    Wp = np.asarray(inputs["Wp"], np.float32)
    bq = np.asarray(inputs["bq"], np.float32)
    bk = np.asarray(inputs["bk"], np.float32)
    ones = np.ones((128, 512), f8).view(np.uint8)
    in_maps = []
    for core in range(NCORES):
        b, g = core // 2, core % 2
        sl = slice(FC * g, FC * (g + 1))
        in_maps.append(
            {
                "xT": np.ascontiguousarray(
                    x[b].T.reshape(KT8, 128, NCQ, 512)
                    .transpose(2, 1, 0, 3)
                    .reshape(NCQ, 128, KT8 * 512)
                    .astype(mdt)
                ),
                "wq": _wmerge_pm(Wq[:, sl], mdt),
                "wk": _wmerge_pm(Wk[:, sl], mdt),
                "wv": _wmerge(Wv[:, sl], mdt),
                "wp": _wmerge(Wp[sl, :], mdt),
                "bq": np.ascontiguousarray(bq[sl].reshape(NP, 128).T),
                "bk": np.ascontiguousarray(bk[sl].reshape(NP, 128).T),
                "ones": ones,
            }
        )
    return in_maps


def assemble(results, inputs):
    Wp = np.asarray(inputs["Wp"], np.float32)
    bv = np.asarray(inputs["bv"], np.float32)
    bp = np.asarray(inputs["bp"], np.float32)
    fb = (bp.astype(np.float64) + bv.astype(np.float64) @ Wp.astype(np.float64)).astype(
        np.float32
    )
    out = np.empty((B, N, C), np.float32)
    for b in range(B):
        yt = (
            results[2 * b]["yT"].astype(np.float32)
            + results[2 * b + 1]["yT"].astype(np.float32)
        ).transpose(0, 2, 1, 3)
        out[b] = yt.reshape(C, N).T + fb
    return out


def run_on_device(inputs, trace=False, tmpdir=None):
    from concourse.bass_utils import run_bass_kernel_spmd

    nc = _get_nc()
    res = run_bass_kernel_spmd(
        nc, make_in_maps(inputs), list(range(NCORES)), trace=trace, tmpdir=tmpdir
    )
    return assemble(res.results, inputs), res


def kernel(**inputs):
    out, _ = run_on_device(inputs)
    return out


# revision 28
# speedup vs baseline: 1.0071x; 1.0071x over previous
"""Multi-head attention (B=4, N=2048, C=1024, H=16, D=64) on 8 trn2 cores.

Sharding: core c handles batch b = c//2 and head-group g = c%2 (8 heads,
512 channels). No collectives: the row-parallel output projection partials
are summed on host (2 cores per batch), with bp + bv@Wp folded in on host
(softmax rows sum to 1, so the v-bias contributes a constant per channel).

Device layout is transposed end-to-end (tokens on the free axis):
  xT [C, N] -> Q^T/K^T pair tiles [128, N] (rows 0:64 head 2p, 64:128 head
  2p+1) -> S^T tiles [keys, queries] via K=64 matmuls -> exp on ACT (no max
  subtraction; scores are O(1) by construction) -> exp output is fp8e4 and
  PV runs in fp8 DoubleRow over key-tile PAIRS (contraction 256): V_aug
  per pair tile [vA | ones64 | vB] x {k=0,1} interleaved; psum rows split
  into O_u and a 64-row replicated rowsum -> DVE reciprocal * mul -> O^T ->
  Y^T = Wp_g^T O^T (bf16).

Schedule: the kernel is ACT(exp)-bound (256 exps x ~1.07us = 274us); the
whole schedule aims to keep ACT 100% busy from ~10us on. Prologue is only
wq+x0+wk DMA and the pair-0 chunk-0 Q/K projection; all other projections
(V for all 16 key tiles, remaining Q/K chunks, later pairs, output
projection) are deadline-scheduled as PE filler inside the attention
strips. A dummy-matmul burst at t0 (on the resident ones tile) keeps the
PE HAM clock gate warm through the initial DMA wait so the first real
matmuls run at 2.4 GHz.

Matmul operand dtypes: bf16 for projections and S (accuracy-critical),
fp8e4m3 for the PV DoubleRow pair (E = exp output, V_aug); accumulation
is always fp32 in PSUM. Measured fro rel err ~1.4e-2 (gate 2e-2).
"""

import os
import sys

sys.path.insert(0, "/opt/trn_rl_repo")

import numpy as np

B, N, C, H = 4, 2048, 1024, 16
D = C // H
SCALE = D**-0.5
NCORES = 8
FC = 512  # channels per core
NP = 4  # head pairs per core
KT8 = C // 128  # contraction tiles
NCQ = N // 512  # n-chunks of 512
NMT = N // 128  # key tiles
NKP = NMT // 2  # key-tile pairs (DoubleRow)

_nc = None


def _cap(ap_slice, block_step, nblocks, width):
    """2-free-dim AP: nblocks blocks of `width` cols, stride block_step."""
    import concourse.bass as bass

    lst = [list(p) for p in ap_slice.ap]
    assert len(lst) == 2 and lst[1][0] == 1, lst
    return bass.AP(
        ap_slice.tensor, ap_slice.offset, [lst[0], [block_step, nblocks], [1, width]]
    )


def _ap3(ap_slice, d1_step, d1_n, d2_step, d2_n):
    """3-free-dim AP for DoubleRow operands: [part, [d1_step,d1_n], [d2_step,d2_n]]."""
    import concourse.bass as bass

    lst = [list(p) for p in ap_slice.ap]
    return bass.AP(
        ap_slice.tensor, ap_slice.offset, [lst[0], [d1_step, d1_n], [d2_step, d2_n]]
    )


def _build():
    import concourse.bacc as bacc
    import concourse.mybir as mybir
    import concourse.tile as tile

    F32 = mybir.dt.float32
    BF16 = mybir.dt.bfloat16
    F8 = mybir.dt.float8e4
    AF = mybir.ActivationFunctionType
    DR = mybir.MatmulPerfMode.DoubleRow

    nc = bacc.Bacc("TRN2", target_bir_lowering=False, debug=False, num_devices=NCORES)

    xT_d = nc.dram_tensor("xT", (NCQ, 128, KT8 * 512), BF16, kind="ExternalInput").ap()
    # wq/wk are pair-major [128, NP, KT8, 128] so pair 0's slices can DMA
    # first; wv keeps the k-major merged layout (its matmuls read full rows).
    wq_d = nc.dram_tensor("wq", (128, KT8 * FC), BF16, kind="ExternalInput").ap()
    wk_d = nc.dram_tensor("wk", (128, KT8 * FC), BF16, kind="ExternalInput").ap()
    wv_d = nc.dram_tensor("wv", (128, KT8 * FC), BF16, kind="ExternalInput").ap()
    wp_d = nc.dram_tensor("wp", (128, NP * C), BF16, kind="ExternalInput").ap()
    bq_d = nc.dram_tensor("bq", (128, NP), F32, kind="ExternalInput").ap()
    bk_d = nc.dram_tensor("bk", (128, NP), F32, kind="ExternalInput").ap()
    on_d = nc.dram_tensor("ones", (128, 512), F8, kind="ExternalInput").ap()
    yT_d = nc.dram_tensor("yT", (C // 128, NCQ, 128, 512), BF16, kind="ExternalOutput").ap()

    with tile.TileContext(nc) as tc:
        with (
            tc.tile_pool(name="sb", bufs=1) as sb,
            tc.tile_pool(name="pe_", bufs=3) as pbe,
            tc.tile_pool(name="prc", bufs=4) as prc,
            tc.tile_pool(name="pyb", bufs=4) as pyb,
            tc.tile_pool(name="psqk", bufs=2, space="PSUM") as psqk,
            tc.tile_pool(name="psa", bufs=2, space="PSUM") as psa,
            tc.tile_pool(name="pso", bufs=2, space="PSUM") as pso,
        ):
            # ---- resident tiles + DMAs ----
            QT = [sb.tile([128, N], BF16, name=f"qt{p}") for p in range(NP)]
            KT = [sb.tile([128, N], BF16, name=f"kt{p}") for p in range(NP)]
            # V_aug DoubleRow tiles: per key-tile pair t, layout
            # [128 keys, (2p+k)*192 + {vA 0:64 | ones 64:128 | vB 128:192}]
            VA = [sb.tile([128, 192 * NP * 2], F8, name=f"va{t}") for t in range(NKP)]
            OT = [sb.tile([128, N], BF16, name=f"ot{p}") for p in range(NP)]
            bq_t = sb.tile([128, NP], F32, name="bq_t")
            bk_t = sb.tile([128, NP], F32, name="bk_t")
            on_t = sb.tile([128, 512], F8, name="on_t")
            z_t = sb.tile([128, 128], BF16, name="z_t")
            nc.sync.dma_start(out=bq_t[:], in_=bq_d)
            nc.sync.dma_start(out=bk_t[:], in_=bk_d)
            nc.sync.dma_start(out=on_t[:], in_=on_d)
            nc.vector.memset(z_t[:], 0.0)

            # ---- HAM warm-up at t0: dummy matmuls on the ones tile keep the
            # PE activity monitor busy through the wq/x0 DMA wait so the
            # first projection matmuls run at 2.4 GHz.
            warm = psqk.tile([128, 512], F32, tag="qk", name="warm")
            for _ in range(16):
                nc.tensor.matmul(
                    warm[:], on_t[:, 0:128], on_t[:],
                    start=True, stop=True, skip_group_check=True,
                )

            # DMA priority: wq/wk pair-0 slices + xT chunk 0 first (gates the
            # first exp), wv next (V filler for the first strip), then the
            # remaining pairs' wq/wk, remaining xT, wp last.
            PW = KT8 * 128  # per-pair width in the pair-major wq/wk layout
            wq_all = sb.tile([128, KT8 * FC], BF16, name="wq_all")
            wk_all = sb.tile([128, KT8 * FC], BF16, name="wk_all")
            wv_all = sb.tile([128, KT8 * FC], BF16, name="wv_all")
            # The DMA path delivers only ~120 GB/s regardless of issue
            # pattern (latency-bound SDMA round-trips), so the x chunks are
            # simply ordered by first use; each is split in column halves so
            # low-k matmuls can start before the full chunk lands.
            HW2 = KT8 * 256
            nc.sync.dma_start(out=wq_all[:, 0:PW], in_=wq_d[:, 0:PW])
            nc.sync.dma_start(out=wk_all[:, 0:PW], in_=wk_d[:, 0:PW])

            def _xall(ncq):
                t = sb.tile([128, KT8 * 512], BF16, name=f"xt_{ncq}")
                nc.sync.dma_start(out=t[:, 0:HW2], in_=xT_d[ncq, :, 0:HW2])
                nc.sync.dma_start(out=t[:, HW2:], in_=xT_d[ncq, :, HW2:])
                return t

            xt_all = [_xall(0), _xall(1)]
            nc.sync.dma_start(out=wv_all[:, 0:HW2], in_=wv_d[:, 0:HW2])
            nc.sync.dma_start(out=wv_all[:, HW2:], in_=wv_d[:, HW2:])
            xt_all.append(_xall(2))
            xt_all.append(_xall(3))
            # Later pairs' weights and wp are DMA'd from inside the step
            # loop (the HWDGE queues interleave all pending transfers, so
            # issuing everything at t0 starves the chunks the first strip
            # is already waiting on). A dummy DVE read of the destination
            # creates a WAR hazard that holds the trigger back to the wall
            # time of the step it is emitted at.
            wp_all = sb.tile([128, NP * C], BF16, name="wp_all")
            gate_t = sb.tile([128, 1], BF16, name="gate_t")

            def emit_w_dma(p):
                def go():
                    nc.vector.tensor_copy(gate_t[:], wq_all[:, PW * p : PW * p + 1])
                    nc.sync.dma_start(out=wq_all[:, PW * p : PW * (p + 1)],
                                      in_=wq_d[:, PW * p : PW * (p + 1)])
                    nc.vector.tensor_copy(gate_t[:], wk_all[:, PW * p : PW * p + 1])
                    nc.sync.dma_start(out=wk_all[:, PW * p : PW * (p + 1)],
                                      in_=wk_d[:, PW * p : PW * (p + 1)])
                return go

            def emit_wp_dma():
                nc.vector.tensor_copy(gate_t[:], wp_all[:, 0:1])
                nc.sync.dma_start(out=wp_all[:], in_=wp_d)

            # ones-fill of the VA DoubleRow tiles (DVE, doesn't touch PE):
            # blocks (2p+k)*192 + 64:128 for all 8 (p,k) -> stride 192 x 8.
            for t in range(NKP):
                nc.vector.tensor_copy(
                    _ap3(VA[t][:, 64:65], 192, NP * 2, 1, 64), on_t[:]
                )

            # ---- QKV emission helpers ----
            def emit_qk_group(p, proj, ncq):
                """One 8-matmul psum group (+ DVE bias evac) for pair p.
                Returns list of closures emitting one instruction each."""
                w_all, bias_t, dst = (
                    (wq_all, bq_t, QT) if proj == 0 else (wk_all, bk_t, KT)
                )
                cs = slice(512 * ncq, 512 * (ncq + 1))
                state = {}

                def mk_mm(k):
                    def go():
                        if "pq" not in state:
                            state["pq"] = psqk.tile(
                                [128, 512], F32, tag="qk", name=f"pq_{p}_{proj}_{ncq}"
                            )
                        nc.tensor.matmul(
                            state["pq"][:],
                            w_all[:, PW * p + 128 * k : PW * p + 128 * (k + 1)],
                            xt_all[ncq][:, 512 * k : 512 * (k + 1)],
                            start=(k == 0), stop=(k == KT8 - 1), skip_group_check=True,
                        )

                    return go

                def evac():
                    nc.vector.tensor_scalar_add(
                        dst[p][:, cs], state["pq"][:], bias_t[:, p : p + 1]
                    )

                return [mk_mm(k) for k in range(KT8)] + [evac]

            def emit_v_group(nt):
                """V projection for key tile nt -> VA[nt//2] slot k=nt%2."""
                ncq, tt = divmod(nt, 4)
                t, k = divmod(nt, 2)
                state = {}

                def mk_mm(kk):
                    def go():
                        if "pv" not in state:
                            state["pv"] = psqk.tile(
                                [128, 512], F32, tag="qk", name=f"pv_{nt}"
                            )
                        nc.tensor.matmul(
                            state["pv"][:],
                            xt_all[ncq][:, 512 * kk + 128 * tt : 512 * kk + 128 * (tt + 1)],
                            wv_all[:, FC * kk : FC * (kk + 1)],
                            start=(kk == 0), stop=(kk == KT8 - 1), skip_group_check=True,
                        )

                    return go

                def evac():
                    va = VA[t]
                    pv = state["pv"]
                    base = 192 * k
                    nc.vector.tensor_copy(
                        _ap3(va[:, base : base + 1], 384, NP, 1, 64),
                        _cap(pv[:, 0:64], 128, NP, 64),
                    )
                    nc.vector.tensor_copy(
                        _ap3(va[:, base + 128 : base + 129], 384, NP, 1, 64),
                        _cap(pv[:, 64:128], 128, NP, 64),
                    )

                return [mk_mm(kk) for kk in range(KT8)] + [evac]

            # ---- output projection chunk emitter ----
            def emit_proj_group(c, ncq, fs=range(NP), partial_out=None, partial_in=None):
                """Y^T chunk: accumulating matmuls over pairs `fs`, then
                either stash the partial (partial_out) or add the stashed
                partial (partial_in) during the bf16 evac + DMA out."""
                cs = slice(512 * ncq, 512 * (ncq + 1))
                fs = list(fs)
                state = {}

                def mk_mm(f):
                    def go():
                        if "py" not in state:
                            state["py"] = psqk.tile(
                                [128, 512], F32, tag="qk", name=f"py_{c}_{ncq}_{fs[0]}"
                            )
                        nc.tensor.matmul(
                            state["py"][:],
                            wp_all[:, C * f + 128 * c : C * f + 128 * (c + 1)],
                            OT[f][:, cs],
                            start=(f == fs[0]), stop=(f == fs[-1]), skip_group_check=True,
                        )

                    return go

                def evac():
                    if partial_out is not None:
                        nc.vector.tensor_copy(partial_out[:], state["py"][:])
                        return
                    yb = pyb.tile([128, 512], BF16, tag="yb", name=f"yb_{c}_{ncq}")
                    if partial_in is not None:
                        nc.vector.tensor_add(yb[:], state["py"][:], partial_in[:])
                    else:
                        nc.vector.tensor_copy(yb[:], state["py"][:])
                    nc.sync.dma_start(out=yT_d[c, ncq, :, :], in_=yb[:])

                return [mk_mm(f) for f in fs] + [evac]

            # ---- deadline-scheduled filler groups ----
            # Each group's closures are spread EVENLY across steps
            # [avail, deadline] at build time. Even spreading (not
            # earliest-first) matters: draining filler early leaves the PE
            # micro-idling in later strips, which trips the HAM clock gate
            # back to 1.2 GHz and the whole pipeline slows ~1.5x.
            NSTEP = NP * NCQ * NMT
            emit_at = [[] for _ in range(NSTEP)]
            drain = []

            def add_group(avail, deadline, closures):
                closures = list(closures)
                if avail >= NSTEP:
                    drain.extend(closures)
                    return
                a = max(avail, 0)
                d = min(max(deadline, a), NSTEP - 1)
                span = d - a
                for j, c in enumerate(closures):
                    s = a + min(span, (j * (span + 1)) // len(closures))
                    emit_at[s].append(c)

            # V projections: VA pair t needed by PV at step 2t+1. avail is
            # aligned with the xT chunk DMA arrival so a stalled V matmul
            # doesn't head-of-line-block the in-order PE queue.
            for nt in range(NMT):
                av = 0 if nt < 4 else 4 * (nt // 4) - 1
                add_group(av, max(nt, 1), emit_v_group(nt))
            # pair-0 remaining Q/K chunks (chunk 0 is the prologue):
            for cq in range(1, NCQ):
                add_group(4 * cq - 3, 4 * cq - 2, emit_qk_group(0, 1, cq))  # K
                add_group(4 * cq - 1, 16 * cq - 2, emit_qk_group(0, 0, cq))  # Q
            # later pairs' Q/K (deadline 64p-2: the S at step 64p is emitted
            # during step 64p-1, so operands must be fully emitted before);
            # their weight DMAs are triggered from inside the loop first:
            add_group(14, 14, [emit_w_dma(1)])
            add_group(38, 38, [emit_w_dma(2), emit_w_dma(3)])
            add_group(60, 60, [emit_wp_dma])
            for p in range(1, NP):
                base = 64 * p
                av = max(18, base - 56)
                for cq in range(NCQ):
                    add_group(av, base + 4 * cq - 2, emit_qk_group(p, 1, cq))
                    add_group(av, base + 16 * cq - 2, emit_qk_group(p, 0, cq))
            # output projection for qc: available after strip (p3, qc) ends;
            # staggered avail so the 8 c-groups don't all land on the same
            # steps, spread all the way to the end. The LAST chunk (qc3)
            # is split: pairs 0..2 accumulate into an SBUF partial during
            # pair-3's strips (their OT rows are long finished); only the
            # f=3 matmul + DVE add + DMA remain after the last step.
            for cq in range(NCQ - 1):
                av = 192 + 16 * cq + 16
                for c in range(C // 128):
                    add_group(av + c, NSTEP - 1, emit_proj_group(c, cq))
            yp = [
                sb.tile([128, 512], BF16, name=f"yp_{c}") for c in range(C // 128)
            ]
            for c in range(C // 128):
                add_group(
                    193 + 2 * c, 248,
                    emit_proj_group(c, 3, fs=range(NP - 1), partial_out=yp[c]),
                )
                drain.extend(
                    emit_proj_group(c, 3, fs=[NP - 1], partial_in=yp[c])
                )

            # ---- prologue: pair-0 chunk-0 Q/K only ----
            for proj in range(2):
                for go in emit_qk_group(0, proj, 0):
                    go()

            # ---- attention strips ----
            # strip = (pair, 512-query-chunk). Both heads of the pair share
            # one [128, 1024] S psum tile: head A (rows 0:64 of K^T/Q^T) ->
            # cols 0:512, head B (rows 64:128) -> cols 512:1024; the two K=64
            # matmuls row-pack onto disjoint PE row-groups and run
            # concurrently. One exp covers both heads and writes fp8 into
            # half of the current E pair tile; PV fires every second step as
            # two DoubleRow matmuls over the key-tile pair (contraction 256).
            # Software-pipelined: S(i+1) is emitted before PV(i//2).
            steps = [(p, qc, mt) for p in range(NP) for qc in range(NCQ) for mt in range(NMT)]

            def emit_S(p, qc, mt):
                qs = slice(512 * qc, 512 * (qc + 1))
                ms = slice(128 * mt, 128 * (mt + 1))
                sa = psa.tile([128, 1024], F32, tag="sa", name=f"sa_{p}_{qc}_{mt}")
                nc.tensor.matmul(
                    sa[:, 0:512], KT[p][0:64, ms], QT[p][0:64, qs],
                    start=True, stop=True,
                )
                nc.tensor.matmul(
                    sa[:, 512:1024], KT[p][64:128, ms], QT[p][64:128, qs],
                    start=True, stop=True,
                )
                return sa

            ots = None
            et = None
            sa_next = emit_S(*steps[0])
            for i, (p, qc, mt) in enumerate(steps):
                if mt == 0:
                    ots = [
                        pso.tile([128, 512], F32, tag="o", name=f"o_{p}_{qc}_{j}")
                        for j in range(2)
                    ]
                if mt % 2 == 0:
                    et = pbe.tile([128, 2048], F8, tag="e", name=f"e_{p}_{qc}_{mt}")
                sa_cur = sa_next
                nc.scalar.activation(
                    et[:, 1024 * (mt % 2) : 1024 * (mt % 2) + 1024],
                    sa_cur[:], AF.Exp, scale=SCALE,
                )
                if i + 1 < len(steps):
                    sa_next = emit_S(*steps[i + 1])
                for go in emit_at[i]:
                    go()
                # pad the PE with zero-weight matmuls accumulating +0 into the
                # live O tile whenever filler is scarce: micro-idle would trip
                # the HAM clock gate back to 1.2 GHz. Skipped at mt<2: there
                # the O tile is freshly pool-rotated and a write would stall
                # the PE behind the previous strip's DVE writeback.
                if mt >= 2:
                    for _ in range(max(0, 2 - len(emit_at[i]))):
                        nc.tensor.matmul(
                            ots[1], z_t[:], QT[p][:, 0:512],
                            start=False, stop=False, skip_group_check=True,
                        )
                if mt % 2 == 1:
                    t = mt // 2
                    first, last = t == 0, t == NKP - 1
                    va = VA[t]
                    nc.tensor.matmul(
                        ots[0],
                        _ap3(va[:, 384 * p : 384 * p + 1], 192, 2, 1, 128),
                        _ap3(et[:, 0:1], 1024, 2, 1, 512),
                        start=first, stop=last, skip_group_check=True,
                        perf_mode=DR,
                    )
                    nc.tensor.matmul(
                        ots[1],
                        _ap3(va[:, 384 * p + 64 : 384 * p + 65], 192, 2, 1, 128),
                        _ap3(et[:, 512:513], 1024, 2, 1, 512),
                        start=first, stop=last, skip_group_check=True,
                        perf_mode=DR,
                    )
                if mt == NMT - 1:
                    qs = slice(512 * qc, 512 * (qc + 1))
                    for j in range(2):
                        o = ots[j]
                        # reciprocal_approx_fast mis-executes at base partition
                        # != 0: run it over the whole tile (unused rows produce
                        # garbage that is never read) and slice after.
                        rc = prc.tile([128, 512], F32, tag="rc", name=f"rc_{p}_{qc}_{j}")
                        nc.vector.reciprocal_approx_fast(rc[:], o[:])
                        osl, rcl = (
                            (o[0:64, :], rc[64:128, :]) if j == 0 else (o[64:128, :], rc[0:64, :])
                        )
                        nc.vector.tensor_mul(OT[p][64 * j : 64 * j + 64, qs], osl, rcl)
            # drain remaining fillers (final output projection chunks)
            for go in drain:
                go()

    nc.compile()
    return nc


def _get_nc():
    global _nc
    if _nc is None:
        try:
            import jax

            jax.config.update(
                "jax_compilation_cache_dir", os.path.expanduser("~/.cache/jax_bass")
            )
            jax.config.update("jax_persistent_cache_min_compile_time_secs", 0.0)
            jax.config.update("jax_persistent_cache_min_entry_size_bytes", 0)
        except Exception:
            pass
        _nc = _build()
    return _nc


def _wmerge(w, mdt):
    """(KT*128, F) -> [128, KT*F] partition-major merged layout."""
    kt = w.shape[0] // 128
    return np.ascontiguousarray(
        w.reshape(kt, 128, w.shape[1]).transpose(1, 0, 2).reshape(128, kt * w.shape[1]).astype(mdt)
    )


def _wmerge_pm(w, mdt):
    """(KT*128, NP*128) -> [128, NP*KT*128] pair-major merged layout:
    out[r, (p*KT + k)*128 + m] = w[128k + r, 128p + m]."""
    kt = w.shape[0] // 128
    npairs = w.shape[1] // 128
    return np.ascontiguousarray(
        w.reshape(kt, 128, npairs, 128)
        .transpose(1, 2, 0, 3)
        .reshape(128, npairs * kt * 128)
        .astype(mdt)
    )


def make_in_maps(inputs):
    import ml_dtypes

    mdt = ml_dtypes.bfloat16
    f8 = ml_dtypes.float8_e4m3fn
    x = np.asarray(inputs["x"], np.float32)
    Wq = np.asarray(inputs["Wq"], np.float32)
    Wk = np.asarray(inputs["Wk"], np.float32)
    Wv = np.asarray(inputs["Wv"], np.float32)
    Wp = np.asarray(inputs["Wp"], np.float32)
    bq = np.asarray(inputs["bq"], np.float32)
    bk = np.asarray(inputs["bk"], np.float32)
    ones = np.ones((128, 512), f8).view(np.uint8)
    in_maps = []
    for core in range(NCORES):
        b, g = core // 2, core % 2
        sl = slice(FC * g, FC * (g + 1))
        in_maps.append(
            {
                "xT": np.ascontiguousarray(
                    x[b].T.reshape(KT8, 128, NCQ, 512)
                    .transpose(2, 1, 0, 3)
                    .reshape(NCQ, 128, KT8 * 512)
                    .astype(mdt)
                ),
                "wq": _wmerge_pm(Wq[:, sl], mdt),
                "wk": _wmerge_pm(Wk[:, sl], mdt),
                "wv": _wmerge(Wv[:, sl], mdt),
                "wp": _wmerge(Wp[sl, :], mdt),
                "bq": np.ascontiguousarray(bq[sl].reshape(NP, 128).T),
                "bk": np.ascontiguousarray(bk[sl].reshape(NP, 128).T),
                "ones": ones,
            }
        )
    return in_maps


def assemble(results, inputs):
    Wp = np.asarray(inputs["Wp"], np.float32)
    bv = np.asarray(inputs["bv"], np.float32)
    bp = np.asarray(inputs["bp"], np.float32)
    fb = (bp.astype(np.float64) + bv.astype(np.float64) @ Wp.astype(np.float64)).astype(
        np.float32
    )
    out = np.empty((B, N, C), np.float32)
    for b in range(B):
        yt = (
            results[2 * b]["yT"].astype(np.float32)
            + results[2 * b + 1]["yT"].astype(np.float32)
        ).transpose(0, 2, 1, 3)
        out[b] = yt.reshape(C, N).T + fb
    return out


def run_on_device(inputs, trace=False, tmpdir=None):
    from concourse.bass_utils import run_bass_kernel_spmd

    nc = _get_nc()
    res = run_bass_kernel_spmd(
        nc, make_in_maps(inputs), list(range(NCORES)), trace=trace, tmpdir=tmpdir
    )
    return assemble(res.results, inputs), res


def kernel(**inputs):
    out, _ = run_on_device(inputs)
    return out


# revision 33
# speedup vs baseline: 1.0172x; 1.0100x over previous
"""Multi-head attention (B=4, N=2048, C=1024, H=16, D=64) on 8 trn2 cores.

Sharding: core c handles batch b = c//2 and head-group g = c%2 (8 heads,
512 channels). No collectives: the row-parallel output projection partials
are summed on host (2 cores per batch), with bp + bv@Wp folded in on host
(softmax rows sum to 1, so the v-bias contributes a constant per channel).

Device layout is transposed end-to-end (tokens on the free axis):
  xT [C, N] -> Q^T/K^T pair tiles [128, N] (rows 0:64 head 2p, 64:128 head
  2p+1) -> S^T tiles [keys, queries] via K=64 matmuls -> exp on ACT (no max
  subtraction; scores are O(1) by construction) -> exp output is fp8e4 and
  PV runs in fp8 DoubleRow over key-tile PAIRS (contraction 256): V_aug
  per pair tile [vA | ones64 | vB] x {k=0,1} interleaved; psum rows split
  into O_u and a 64-row replicated rowsum -> DVE reciprocal * mul -> O^T ->
  Y^T = Wp_g^T O^T (bf16).

Schedule: the kernel is ACT(exp)-bound (256 exps x ~1.07us = 274us); the
whole schedule aims to keep ACT 100% busy from ~10us on. Prologue is only
wq+x0+wk DMA and the pair-0 chunk-0 Q/K projection; all other projections
(V for all 16 key tiles, remaining Q/K chunks, later pairs, output
projection) are deadline-scheduled as PE filler inside the attention
strips. A dummy-matmul burst at t0 (on the resident ones tile) keeps the
PE HAM clock gate warm through the initial DMA wait so the first real
matmuls run at 2.4 GHz.

Matmul operand dtypes: bf16 for projections and S (accuracy-critical),
fp8e4m3 for the PV DoubleRow pair (E = exp output, V_aug); accumulation
is always fp32 in PSUM. Measured fro rel err ~1.4e-2 (gate 2e-2).
"""

import os
import sys

sys.path.insert(0, "/opt/trn_rl_repo")

import numpy as np

B, N, C, H = 4, 2048, 1024, 16
D = C // H
SCALE = D**-0.5
NCORES = 8
FC = 512  # channels per core
NP = 4  # head pairs per core
KT8 = C // 128  # contraction tiles
NCQ = N // 512  # n-chunks of 512
NMT = N // 128  # key tiles
NKP = NMT // 2  # key-tile pairs (DoubleRow)

_nc = None


def _cap(ap_slice, block_step, nblocks, width):
    """2-free-dim AP: nblocks blocks of `width` cols, stride block_step."""
    import concourse.bass as bass

    lst = [list(p) for p in ap_slice.ap]
    assert len(lst) == 2 and lst[1][0] == 1, lst
    return bass.AP(
        ap_slice.tensor, ap_slice.offset, [lst[0], [block_step, nblocks], [1, width]]
    )


def _ap3(ap_slice, d1_step, d1_n, d2_step, d2_n):
    """3-free-dim AP for DoubleRow operands: [part, [d1_step,d1_n], [d2_step,d2_n]]."""
    import concourse.bass as bass

    lst = [list(p) for p in ap_slice.ap]
    return bass.AP(
        ap_slice.tensor, ap_slice.offset, [lst[0], [d1_step, d1_n], [d2_step, d2_n]]
    )


def _build():
    import concourse.bacc as bacc
    import concourse.mybir as mybir
    import concourse.tile as tile

    F32 = mybir.dt.float32
    BF16 = mybir.dt.bfloat16
    F8 = mybir.dt.float8e4
    AF = mybir.ActivationFunctionType
    DR = mybir.MatmulPerfMode.DoubleRow

    nc = bacc.Bacc("TRN2", target_bir_lowering=False, debug=False, num_devices=NCORES)

    xT_d = nc.dram_tensor("xT", (NCQ, 128, KT8 * 512), BF16, kind="ExternalInput").ap()
    # wq/wk are pair-major [128, NP, KT8, 128] so pair 0's slices can DMA
    # first; wv keeps the k-major merged layout (its matmuls read full rows).
    wq_d = nc.dram_tensor("wq", (128, KT8 * FC), BF16, kind="ExternalInput").ap()
    wk_d = nc.dram_tensor("wk", (128, KT8 * FC), BF16, kind="ExternalInput").ap()
    wv_d = nc.dram_tensor("wv", (128, KT8 * FC), BF16, kind="ExternalInput").ap()
    wp_d = nc.dram_tensor("wp", (128, NP * C), BF16, kind="ExternalInput").ap()
    bq_d = nc.dram_tensor("bq", (128, NP), F32, kind="ExternalInput").ap()
    bk_d = nc.dram_tensor("bk", (128, NP), F32, kind="ExternalInput").ap()
    on_d = nc.dram_tensor("ones", (128, 512), F8, kind="ExternalInput").ap()
    yT_d = nc.dram_tensor("yT", (C // 128, NCQ, 128, 512), BF16, kind="ExternalOutput").ap()

    with tile.TileContext(nc) as tc:
        with (
            tc.tile_pool(name="sb", bufs=1) as sb,
            tc.tile_pool(name="pe_", bufs=3) as pbe,
            tc.tile_pool(name="prc", bufs=4) as prc,
            tc.tile_pool(name="pyb", bufs=4) as pyb,
            tc.tile_pool(name="psqk", bufs=2, space="PSUM") as psqk,
            tc.tile_pool(name="psa", bufs=2, space="PSUM") as psa,
            tc.tile_pool(name="pso", bufs=2, space="PSUM") as pso,
        ):
            # ---- resident tiles + DMAs ----
            QT = [sb.tile([128, N], BF16, name=f"qt{p}") for p in range(NP)]
            KT = [sb.tile([128, N], BF16, name=f"kt{p}") for p in range(NP)]
            # V_aug DoubleRow tiles: per key-tile pair t, layout
            # [128 keys, (2p+k)*192 + {vA 0:64 | ones 64:128 | vB 128:192}]
            VA = [sb.tile([128, 192 * NP * 2], F8, name=f"va{t}") for t in range(NKP)]
            OT = [sb.tile([128, N], BF16, name=f"ot{p}") for p in range(NP)]
            bq_t = sb.tile([128, NP], F32, name="bq_t")
            bk_t = sb.tile([128, NP], F32, name="bk_t")
            on_t = sb.tile([128, 512], F8, name="on_t")
            z_t = sb.tile([128, 128], BF16, name="z_t")
            nc.sync.dma_start(out=bq_t[:], in_=bq_d)
            nc.sync.dma_start(out=bk_t[:], in_=bk_d)
            nc.sync.dma_start(out=on_t[:], in_=on_d)
            nc.vector.memset(z_t[:], 0.0)

            # ---- HAM warm-up at t0: dummy matmuls on the ones tile keep the
            # PE activity monitor busy through the wq/x0 DMA wait so the
            # first projection matmuls run at 2.4 GHz.
            warm = psqk.tile([128, 512], F32, tag="qk", name="warm")
            for _ in range(16):
                nc.tensor.matmul(
                    warm[:], on_t[:, 0:128], on_t[:],
                    start=True, stop=True, skip_group_check=True,
                )

            # DMA priority: wq/wk pair-0 slices + xT chunk 0 first (gates the
            # first exp), wv next (V filler for the first strip), then the
            # remaining pairs' wq/wk, remaining xT, wp last.
            PW = KT8 * 128  # per-pair width in the pair-major wq/wk layout
            wq_all = sb.tile([128, KT8 * FC], BF16, name="wq_all")
            wk_all = sb.tile([128, KT8 * FC], BF16, name="wk_all")
            wv_all = sb.tile([128, KT8 * FC], BF16, name="wv_all")
            # The DMA path delivers only ~120 GB/s regardless of issue
            # pattern (latency-bound SDMA round-trips), so the x chunks are
            # simply ordered by first use; each is split in column halves so
            # low-k matmuls can start before the full chunk lands.
            HW2 = KT8 * 256
            nc.sync.dma_start(out=wq_all[:, 0:PW], in_=wq_d[:, 0:PW])
            nc.sync.dma_start(out=wk_all[:, 0:PW], in_=wk_d[:, 0:PW])

            def _xall(ncq):
                t = sb.tile([128, KT8 * 512], BF16, name=f"xt_{ncq}")
                nc.sync.dma_start(out=t[:, 0:HW2], in_=xT_d[ncq, :, 0:HW2])
                nc.sync.dma_start(out=t[:, HW2:], in_=xT_d[ncq, :, HW2:])
                return t

            xt_all = [_xall(0), _xall(1)]
            nc.sync.dma_start(out=wv_all[:, 0:HW2], in_=wv_d[:, 0:HW2])
            nc.sync.dma_start(out=wv_all[:, HW2:], in_=wv_d[:, HW2:])
            xt_all.append(_xall(2))
            xt_all.append(_xall(3))
            # Later pairs' weights and wp are DMA'd from inside the step
            # loop (the HWDGE queues interleave all pending transfers, so
            # issuing everything at t0 starves the chunks the first strip
            # is already waiting on). A dummy DVE read of the destination
            # creates a WAR hazard that holds the trigger back to the wall
            # time of the step it is emitted at.
            wp_all = sb.tile([128, NP * C], BF16, name="wp_all")
            gate_t = sb.tile([128, 1], BF16, name="gate_t")

            def emit_w_dma(p):
                def go():
                    nc.vector.tensor_copy(gate_t[:], wq_all[:, PW * p : PW * p + 1])
                    nc.sync.dma_start(out=wq_all[:, PW * p : PW * (p + 1)],
                                      in_=wq_d[:, PW * p : PW * (p + 1)])
                    nc.vector.tensor_copy(gate_t[:], wk_all[:, PW * p : PW * p + 1])
                    nc.sync.dma_start(out=wk_all[:, PW * p : PW * (p + 1)],
                                      in_=wk_d[:, PW * p : PW * (p + 1)])
                return go

            def emit_wp_dma():
                nc.vector.tensor_copy(gate_t[:], wp_all[:, 0:1])
                nc.sync.dma_start(out=wp_all[:], in_=wp_d)

            # ones-fill of the VA DoubleRow tiles (DVE, doesn't touch PE):
            # blocks (2p+k)*192 + 64:128 for all 8 (p,k) -> stride 192 x 8.
            for t in range(NKP):
                nc.vector.tensor_copy(
                    _ap3(VA[t][:, 64:65], 192, NP * 2, 1, 64), on_t[:]
                )

            # ---- QKV emission helpers ----
            def emit_qk_group(p, proj, ncq):
                """One 8-matmul psum group (+ DVE bias evac) for pair p.
                Returns list of closures emitting one instruction each."""
                w_all, bias_t, dst = (
                    (wq_all, bq_t, QT) if proj == 0 else (wk_all, bk_t, KT)
                )
                cs = slice(512 * ncq, 512 * (ncq + 1))
                state = {}

                def mk_mm(k):
                    def go():
                        if "pq" not in state:
                            state["pq"] = psqk.tile(
                                [128, 512], F32, tag="qk", name=f"pq_{p}_{proj}_{ncq}"
                            )
                        nc.tensor.matmul(
                            state["pq"][:],
                            w_all[:, PW * p + 128 * k : PW * p + 128 * (k + 1)],
                            xt_all[ncq][:, 512 * k : 512 * (k + 1)],
                            start=(k == 0), stop=(k == KT8 - 1), skip_group_check=True,
                        )

                    return go

                def evac():
                    nc.vector.tensor_scalar_add(
                        dst[p][:, cs], state["pq"][:], bias_t[:, p : p + 1]
                    )

                return [mk_mm(k) for k in range(KT8)] + [evac]

            def emit_v_group(nt):
                """V projection for key tile nt -> VA[nt//2] slot k=nt%2."""
                ncq, tt = divmod(nt, 4)
                t, k = divmod(nt, 2)
                state = {}

                def mk_mm(kk):
                    def go():
                        if "pv" not in state:
                            state["pv"] = psqk.tile(
                                [128, 512], F32, tag="qk", name=f"pv_{nt}"
                            )
                        nc.tensor.matmul(
                            state["pv"][:],
                            xt_all[ncq][:, 512 * kk + 128 * tt : 512 * kk + 128 * (tt + 1)],
                            wv_all[:, FC * kk : FC * (kk + 1)],
                            start=(kk == 0), stop=(kk == KT8 - 1), skip_group_check=True,
                        )

                    return go

                def evac():
                    va = VA[t]
                    pv = state["pv"]
                    base = 192 * k
                    nc.vector.tensor_copy(
                        _ap3(va[:, base : base + 1], 384, NP, 1, 64),
                        _cap(pv[:, 0:64], 128, NP, 64),
                    )
                    nc.vector.tensor_copy(
                        _ap3(va[:, base + 128 : base + 129], 384, NP, 1, 64),
                        _cap(pv[:, 64:128], 128, NP, 64),
                    )

                return [mk_mm(kk) for kk in range(KT8)] + [evac]

            # ---- output projection chunk emitter ----
            def emit_proj_group(c, ncq, fs=range(NP), partial_out=None, partial_in=None):
                """Y^T chunk: accumulating matmuls over pairs `fs`, then
                either stash the partial (partial_out) or add the stashed
                partial (partial_in) during the bf16 evac + DMA out."""
                cs = slice(512 * ncq, 512 * (ncq + 1))
                fs = list(fs)
                state = {}

                def mk_mm(f):
                    def go():
                        if "py" not in state:
                            state["py"] = psqk.tile(
                                [128, 512], F32, tag="qk", name=f"py_{c}_{ncq}_{fs[0]}"
                            )
                        nc.tensor.matmul(
                            state["py"][:],
                            wp_all[:, C * f + 128 * c : C * f + 128 * (c + 1)],
                            OT[f][:, cs],
                            start=(f == fs[0]), stop=(f == fs[-1]), skip_group_check=True,
                        )

                    return go

                def evac():
                    if partial_out is not None:
                        nc.vector.tensor_copy(partial_out[:], state["py"][:])
                        return
                    yb = pyb.tile([128, 512], BF16, tag="yb", name=f"yb_{c}_{ncq}")
                    if partial_in is not None:
                        nc.vector.tensor_add(yb[:], state["py"][:], partial_in[:])
                    else:
                        nc.vector.tensor_copy(yb[:], state["py"][:])
                    nc.sync.dma_start(out=yT_d[c, ncq, :, :], in_=yb[:])

                return [mk_mm(f) for f in fs] + [evac]

            # ---- deadline-scheduled filler groups ----
            # Each group's closures are spread EVENLY across steps
            # [avail, deadline] at build time. Even spreading (not
            # earliest-first) matters: draining filler early leaves the PE
            # micro-idling in later strips, which trips the HAM clock gate
            # back to 1.2 GHz and the whole pipeline slows ~1.5x.
            NSTEP = NP * NCQ * NMT
            emit_at = [[] for _ in range(NSTEP)]
            drain = []
            _sched_groups = []

            def add_group(avail, deadline, closures, psum=True):
                closures = list(closures)
                if avail >= NSTEP:
                    drain.extend(closures)
                    return
                _sched_groups.append(
                    {"avail": max(avail, 0), "dl": min(max(deadline, 0), NSTEP - 1),
                     "cl": closures, "psum": psum}
                )

            def assign_fillers():
                """Sequential tiler: groups in deadline order, closures packed
                at <=cap/step, bursting only when a deadline forces it. A
                psum group may not start before the second-previous psum
                group's last step — the filler PSUM pool has 2 buffers, and
                3+ concurrently open groups can deadlock the PE against the
                DVE evacuations."""
                load = [0] * NSTEP

                def cap(s):
                    return 12 if s < NMT else (2 if s < 192 else 3)

                _sched_groups.sort(key=lambda g: (g["dl"], g["avail"]))
                open_last = [0, 0]
                for g in _sched_groups:
                    s = max(g["avail"], open_last[0] if g["psum"] else 0)
                    for c in g["cl"]:
                        while s < g["dl"] and load[s] >= cap(s):
                            s += 1
                        emit_at[s].append(c)
                        load[s] += 1
                    if g["psum"]:
                        open_last = [open_last[1], s]

            # V projections: VA pair t needed by PV at step 2t+1. avail is
            # aligned with the xT chunk DMA arrival so a stalled V matmul
            # doesn't head-of-line-block the in-order PE queue.
            for nt in range(NMT):
                av = 0 if nt < 4 else 4 * (nt // 4) - 1
                add_group(av, max(nt, 1), emit_v_group(nt))
            # pair-0 remaining Q/K chunks (chunk 0 is the prologue):
            for cq in range(1, NCQ):
                add_group(4 * cq - 3, 4 * cq - 2, emit_qk_group(0, 1, cq))  # K
                add_group(4 * cq - 1, 16 * cq - 2, emit_qk_group(0, 0, cq))  # Q
            # later pairs' Q/K (deadline 64p-2: the S at step 64p is emitted
            # during step 64p-1, so operands must be fully emitted before);
            # their weight DMAs are triggered from inside the loop first:
            add_group(14, 14, [emit_w_dma(1)], psum=False)
            add_group(38, 38, [emit_w_dma(2), emit_w_dma(3)], psum=False)
            add_group(60, 60, [emit_wp_dma], psum=False)
            for p in range(1, NP):
                base = 64 * p
                av = max(18, base - 56)
                for cq in range(NCQ):
                    add_group(av, base + 4 * cq - 2, emit_qk_group(p, 1, cq))
                    add_group(av, base + 16 * cq - 2, emit_qk_group(p, 0, cq))
            # output projection for qc: available after strip (p3, qc) ends;
            # staggered avail so the 8 c-groups don't all land on the same
            # steps, spread all the way to the end. The LAST chunk (qc3)
            # is split: pairs 0..2 accumulate into an SBUF partial during
            # pair-3's strips (their OT rows are long finished); only the
            # f=3 matmul + DVE add + DMA remain after the last step.
            for cq in range(NCQ - 1):
                av = 192 + 16 * cq + 16
                for c in range(C // 128):
                    add_group(av + c, NSTEP - 1, emit_proj_group(c, cq))
            yp = [
                sb.tile([128, 512], BF16, name=f"yp_{c}") for c in range(C // 128)
            ]
            for c in range(C // 128):
                add_group(
                    193 + 2 * c, 248,
                    emit_proj_group(c, 3, fs=range(NP - 1), partial_out=yp[c]),
                )
                drain.extend(
                    emit_proj_group(c, 3, fs=[NP - 1], partial_in=yp[c])
                )
            assign_fillers()

            # ---- prologue: pair-0 chunk-0 Q/K only ----
            for proj in range(2):
                for go in emit_qk_group(0, proj, 0):
                    go()

            # ---- attention strips ----
            # strip = (pair, 512-query-chunk). Both heads of the pair share
            # one [128, 1024] S psum tile: head A (rows 0:64 of K^T/Q^T) ->
            # cols 0:512, head B (rows 64:128) -> cols 512:1024; the two K=64
            # matmuls row-pack onto disjoint PE row-groups and run
            # concurrently. One exp covers both heads and writes fp8 into
            # half of the current E pair tile; PV fires every second step as
            # two DoubleRow matmuls over the key-tile pair (contraction 256).
            # Software-pipelined: S(i+1) is emitted before PV(i//2).
            steps = [(p, qc, mt) for p in range(NP) for qc in range(NCQ) for mt in range(NMT)]

            def emit_S(p, qc, mt):
                qs = slice(512 * qc, 512 * (qc + 1))
                ms = slice(128 * mt, 128 * (mt + 1))
                sa = psa.tile([128, 1024], F32, tag="sa", name=f"sa_{p}_{qc}_{mt}")
                nc.tensor.matmul(
                    sa[:, 0:512], KT[p][0:64, ms], QT[p][0:64, qs],
                    start=True, stop=True,
                )
                nc.tensor.matmul(
                    sa[:, 512:1024], KT[p][64:128, ms], QT[p][64:128, qs],
                    start=True, stop=True,
                )
                return sa

            ots = None
            et = None
            sa_next = emit_S(*steps[0])
            for i, (p, qc, mt) in enumerate(steps):
                if mt == 0:
                    ots = [
                        pso.tile([128, 512], F32, tag="o", name=f"o_{p}_{qc}_{j}")
                        for j in range(2)
                    ]
                if mt % 2 == 0:
                    et = pbe.tile([128, 2048], F8, tag="e", name=f"e_{p}_{qc}_{mt}")
                sa_cur = sa_next
                nc.scalar.activation(
                    et[:, 1024 * (mt % 2) : 1024 * (mt % 2) + 1024],
                    sa_cur[:], AF.Exp, scale=SCALE,
                )
                if i + 1 < len(steps):
                    sa_next = emit_S(*steps[i + 1])
                for go in emit_at[i]:
                    go()
                # pad the PE with zero-weight matmuls accumulating +0 into the
                # live O tile whenever filler is scarce: micro-idle would trip
                # the HAM clock gate back to 1.2 GHz. Skipped at mt<2: there
                # the O tile is freshly pool-rotated and a write would stall
                # the PE behind the previous strip's DVE writeback.
                if mt >= 2:
                    for _ in range(max(0, 2 - len(emit_at[i]))):
                        nc.tensor.matmul(
                            ots[1], z_t[:], QT[p][:, 0:512],
                            start=False, stop=False, skip_group_check=True,
                        )
                if mt % 2 == 1:
                    t = mt // 2
                    first, last = t == 0, t == NKP - 1
                    va = VA[t]
                    nc.tensor.matmul(
                        ots[0],
                        _ap3(va[:, 384 * p : 384 * p + 1], 192, 2, 1, 128),
                        _ap3(et[:, 0:1], 1024, 2, 1, 512),
                        start=first, stop=last, skip_group_check=True,
                        perf_mode=DR,
                    )
                    nc.tensor.matmul(
                        ots[1],
                        _ap3(va[:, 384 * p + 64 : 384 * p + 65], 192, 2, 1, 128),
                        _ap3(et[:, 512:513], 1024, 2, 1, 512),
                        start=first, stop=last, skip_group_check=True,
                        perf_mode=DR,
                    )
                if mt == NMT - 1:
                    qs = slice(512 * qc, 512 * (qc + 1))
                    for j in range(2):
                        o = ots[j]
                        # reciprocal_approx_fast mis-executes at base partition
                        # != 0: run it over the whole tile (unused rows produce
                        # garbage that is never read) and slice after.
                        rc = prc.tile([128, 512], F32, tag="rc", name=f"rc_{p}_{qc}_{j}")
                        nc.vector.reciprocal_approx_fast(rc[:], o[:])
                        osl, rcl = (
                            (o[0:64, :], rc[64:128, :]) if j == 0 else (o[64:128, :], rc[0:64, :])
                        )
                        nc.vector.tensor_mul(OT[p][64 * j : 64 * j + 64, qs], osl, rcl)
            # drain remaining fillers (final output projection chunks)
            for go in drain:
                go()

    nc.compile()
    return nc


def _get_nc():
    global _nc
    if _nc is None:
        try:
            import jax

            jax.config.update(
                "jax_compilation_cache_dir", os.path.expanduser("~/.cache/jax_bass")
            )
            jax.config.update("jax_persistent_cache_min_compile_time_secs", 0.0)
            jax.config.update("jax_persistent_cache_min_entry_size_bytes", 0)
        except Exception:
            pass
        _nc = _build()
    return _nc


def _wmerge(w, mdt):
    """(KT*128, F) -> [128, KT*F] partition-major merged layout."""
    kt = w.shape[0] // 128
    return np.ascontiguousarray(
        w.reshape(kt, 128, w.shape[1]).transpose(1, 0, 2).reshape(128, kt * w.shape[1]).astype(mdt)
    )


def _wmerge_pm(w, mdt):
    """(KT*128, NP*128) -> [128, NP*KT*128] pair-major merged layout:
    out[r, (p*KT + k)*128 + m] = w[128k + r, 128p + m]."""
    kt = w.shape[0] // 128
    npairs = w.shape[1] // 128
    return np.ascontiguousarray(
        w.reshape(kt, 128, npairs, 128)
        .transpose(1, 2, 0, 3)
        .reshape(128, npairs * kt * 128)
        .astype(mdt)
    )


def make_in_maps(inputs):
    import ml_dtypes

    mdt = ml_dtypes.bfloat16
    f8 = ml_dtypes.float8_e4m3fn
    x = np.asarray(inputs["x"], np.float32)
    Wq = np.asarray(inputs["Wq"], np.float32)
    Wk = np.asarray(inputs["Wk"], np.float32)
    Wv = np.asarray(inputs["Wv"], np.float32)
    Wp = np.asarray(inputs["Wp"], np.float32)
    bq = np.asarray(inputs["bq"], np.float32)
    bk = np.asarray(inputs["bk"], np.float32)
    ones = np.ones((128, 512), f8).view(np.uint8)
    in_maps = []
    for core in range(NCORES):
        b, g = core // 2, core % 2
        sl = slice(FC * g, FC * (g + 1))
        in_maps.append(
            {
                "xT": np.ascontiguousarray(
                    x[b].T.reshape(KT8, 128, NCQ, 512)
                    .transpose(2, 1, 0, 3)
                    .reshape(NCQ, 128, KT8 * 512)
                    .astype(mdt)
                ),
                "wq": _wmerge_pm(Wq[:, sl], mdt),
                "wk": _wmerge_pm(Wk[:, sl], mdt),
                "wv": _wmerge(Wv[:, sl], mdt),
                "wp": _wmerge(Wp[sl, :], mdt),
                "bq": np.ascontiguousarray(bq[sl].reshape(NP, 128).T),
                "bk": np.ascontiguousarray(bk[sl].reshape(NP, 128).T),
                "ones": ones,
            }
        )
    return in_maps


def assemble(results, inputs):
    Wp = np.asarray(inputs["Wp"], np.float32)
    bv = np.asarray(inputs["bv"], np.float32)
    bp = np.asarray(inputs["bp"], np.float32)
    fb = (bp.astype(np.float64) + bv.astype(np.float64) @ Wp.astype(np.float64)).astype(
        np.float32
    )
    out = np.empty((B, N, C), np.float32)
    for b in range(B):
        yt = (
            results[2 * b]["yT"].astype(np.float32)
            + results[2 * b + 1]["yT"].astype(np.float32)
        ).transpose(0, 2, 1, 3)
        out[b] = yt.reshape(C, N).T + fb
    return out


def run_on_device(inputs, trace=False, tmpdir=None):
    from concourse.bass_utils import run_bass_kernel_spmd

    nc = _get_nc()
    res = run_bass_kernel_spmd(
        nc, make_in_maps(inputs), list(range(NCORES)), trace=trace, tmpdir=tmpdir
    )
    return assemble(res.results, inputs), res


def kernel(**inputs):
    out, _ = run_on_device(inputs)
    return out


# revision 36
# speedup vs baseline: 1.0424x; 1.0248x over previous
"""Multi-head attention (B=4, N=2048, C=1024, H=16, D=64) on 8 trn2 cores.

Sharding: core c handles batch b = c//2 and head-group g = c%2 (8 heads,
512 channels). No collectives: the row-parallel output projection partials
are summed on host (2 cores per batch), with bp + bv@Wp folded in on host
(softmax rows sum to 1, so the v-bias contributes a constant per channel).

Device layout is transposed end-to-end (tokens on the free axis):
  xT [C, N] -> Q^T/K^T pair tiles [128, N] (rows 0:64 head 2p, 64:128 head
  2p+1) -> S^T tiles [keys, queries] via K=64 matmuls -> exp on ACT (no max
  subtraction; scores are O(1) by construction) -> exp output is fp8e4 and
  PV runs in fp8 DoubleRow over key-tile PAIRS (contraction 256): V_aug
  per pair tile [vA | ones64 | vB] x {k=0,1} interleaved; psum rows split
  into O_u and a 64-row replicated rowsum -> DVE reciprocal * mul -> O^T ->
  Y^T = Wp_g^T O^T (bf16).

Schedule: the kernel is ACT(exp)-bound (256 exps x ~1.07us = 274us); the
whole schedule aims to keep ACT 100% busy from ~10us on. Prologue is only
wq+x0+wk DMA and the pair-0 chunk-0 Q/K projection; all other projections
(V for all 16 key tiles, remaining Q/K chunks, later pairs, output
projection) are deadline-scheduled as PE filler inside the attention
strips. A dummy-matmul burst at t0 (on the resident ones tile) keeps the
PE HAM clock gate warm through the initial DMA wait so the first real
matmuls run at 2.4 GHz.

Matmul operand dtypes: bf16 for projections and S (accuracy-critical),
fp8e4m3 for the PV DoubleRow pair (E = exp output, V_aug); accumulation
is always fp32 in PSUM. Measured fro rel err ~1.4e-2 (gate 2e-2).
"""

import os
import sys

sys.path.insert(0, "/opt/trn_rl_repo")

import numpy as np

B, N, C, H = 4, 2048, 1024, 16
D = C // H
SCALE = D**-0.5
NCORES = 8
FC = 512  # channels per core
NP = 4  # head pairs per core
KT8 = C // 128  # contraction tiles
NCQ = N // 512  # n-chunks of 512
NMT = N // 128  # key tiles
NKP = NMT // 2  # key-tile pairs (DoubleRow)

_nc = None


def _cap(ap_slice, block_step, nblocks, width):
    """2-free-dim AP: nblocks blocks of `width` cols, stride block_step."""
    import concourse.bass as bass

    lst = [list(p) for p in ap_slice.ap]
    assert len(lst) == 2 and lst[1][0] == 1, lst
    return bass.AP(
        ap_slice.tensor, ap_slice.offset, [lst[0], [block_step, nblocks], [1, width]]
    )


def _ap3(ap_slice, d1_step, d1_n, d2_step, d2_n):
    """3-free-dim AP for DoubleRow operands: [part, [d1_step,d1_n], [d2_step,d2_n]]."""
    import concourse.bass as bass

    lst = [list(p) for p in ap_slice.ap]
    return bass.AP(
        ap_slice.tensor, ap_slice.offset, [lst[0], [d1_step, d1_n], [d2_step, d2_n]]
    )


def _build():
    import concourse.bacc as bacc
    import concourse.mybir as mybir
    import concourse.tile as tile

    F32 = mybir.dt.float32
    BF16 = mybir.dt.bfloat16
    F8 = mybir.dt.float8e4
    AF = mybir.ActivationFunctionType
    DR = mybir.MatmulPerfMode.DoubleRow

    nc = bacc.Bacc("TRN2", target_bir_lowering=False, debug=False, num_devices=NCORES)

    xT_d = nc.dram_tensor("xT", (NCQ, 128, KT8 * 512), BF16, kind="ExternalInput").ap()
    # wq/wk are pair-major [128, NP, KT8, 128] so pair 0's slices can DMA
    # first; wv keeps the k-major merged layout (its matmuls read full rows).
    wq_d = nc.dram_tensor("wq", (128, KT8 * FC), BF16, kind="ExternalInput").ap()
    wk_d = nc.dram_tensor("wk", (128, KT8 * FC), BF16, kind="ExternalInput").ap()
    wv_d = nc.dram_tensor("wv", (128, KT8 * FC), BF16, kind="ExternalInput").ap()
    wp_d = nc.dram_tensor("wp", (128, NP * C), BF16, kind="ExternalInput").ap()
    bq_d = nc.dram_tensor("bq", (128, NP), F32, kind="ExternalInput").ap()
    bk_d = nc.dram_tensor("bk", (128, NP), F32, kind="ExternalInput").ap()
    on_d = nc.dram_tensor("ones", (128, 512), F8, kind="ExternalInput").ap()
    yT_d = nc.dram_tensor("yT", (C // 128, NCQ, 128, 512), BF16, kind="ExternalOutput").ap()

    with tile.TileContext(nc) as tc:
        with (
            tc.tile_pool(name="sb", bufs=1) as sb,
            tc.tile_pool(name="pe_", bufs=3) as pbe,
            tc.tile_pool(name="prc", bufs=4) as prc,
            tc.tile_pool(name="pyb", bufs=4) as pyb,
            tc.tile_pool(name="psqk", bufs=2, space="PSUM") as psqk,
            tc.tile_pool(name="psa", bufs=2, space="PSUM") as psa,
            tc.tile_pool(name="pso", bufs=2, space="PSUM") as pso,
        ):
            # ---- resident tiles + DMAs ----
            QT = [sb.tile([128, N], BF16, name=f"qt{p}") for p in range(NP)]
            KT = [sb.tile([128, N], BF16, name=f"kt{p}") for p in range(NP)]
            # V_aug DoubleRow tiles: per key-tile pair t, layout
            # [128 keys, (2p+k)*192 + {vA 0:64 | ones 64:128 | vB 128:192}]
            VA = [sb.tile([128, 192 * NP * 2], F8, name=f"va{t}") for t in range(NKP)]
            OT = [sb.tile([128, N], BF16, name=f"ot{p}") for p in range(NP)]
            bq_t = sb.tile([128, NP], F32, name="bq_t")
            bk_t = sb.tile([128, NP], F32, name="bk_t")
            on_t = sb.tile([128, 512], F8, name="on_t")
            z_t = sb.tile([128, 128], BF16, name="z_t")
            nc.sync.dma_start(out=bq_t[:], in_=bq_d)
            nc.sync.dma_start(out=bk_t[:], in_=bk_d)
            nc.sync.dma_start(out=on_t[:], in_=on_d)
            nc.vector.memset(z_t[:], 0.0)

            # ---- HAM warm-up at t0: dummy matmuls on the ones tile keep the
            # PE activity monitor busy through the wq/x0 DMA wait so the
            # first projection matmuls run at 2.4 GHz.
            warm = psqk.tile([128, 512], F32, tag="qk", name="warm")
            for _ in range(16):
                nc.tensor.matmul(
                    warm[:], on_t[:, 0:128], on_t[:],
                    start=True, stop=True, skip_group_check=True,
                )

            # DMA priority: wq/wk pair-0 slices + xT chunk 0 first (gates the
            # first exp), wv next (V filler for the first strip), then the
            # remaining pairs' wq/wk, remaining xT, wp last.
            PW = KT8 * 128  # per-pair width in the pair-major wq/wk layout
            wq_all = sb.tile([128, KT8 * FC], BF16, name="wq_all")
            wk_all = sb.tile([128, KT8 * FC], BF16, name="wk_all")
            wv_all = sb.tile([128, KT8 * FC], BF16, name="wv_all")
            # The DMA path delivers only ~120 GB/s regardless of issue
            # pattern (latency-bound SDMA round-trips), so the x chunks are
            # simply ordered by first use; each is split in column halves so
            # low-k matmuls can start before the full chunk lands.
            HW2 = KT8 * 256
            nc.sync.dma_start(out=wq_all[:, 0:PW], in_=wq_d[:, 0:PW])
            nc.sync.dma_start(out=wk_all[:, 0:PW], in_=wk_d[:, 0:PW])

            def _xall(ncq):
                t = sb.tile([128, KT8 * 512], BF16, name=f"xt_{ncq}")
                nc.sync.dma_start(out=t[:, 0:HW2], in_=xT_d[ncq, :, 0:HW2])
                nc.sync.dma_start(out=t[:, HW2:], in_=xT_d[ncq, :, HW2:])
                return t

            xt_all = [_xall(0), _xall(1)]
            nc.sync.dma_start(out=wv_all[:, 0:HW2], in_=wv_d[:, 0:HW2])
            nc.sync.dma_start(out=wv_all[:, HW2:], in_=wv_d[:, HW2:])
            xt_all.append(_xall(2))
            xt_all.append(_xall(3))
            # Later pairs' weights and wp are DMA'd from inside the step
            # loop (the HWDGE queues interleave all pending transfers, so
            # issuing everything at t0 starves the chunks the first strip
            # is already waiting on). A dummy DVE read of the destination
            # creates a WAR hazard that holds the trigger back to the wall
            # time of the step it is emitted at.
            wp_all = sb.tile([128, NP * C], BF16, name="wp_all")
            gate_t = sb.tile([128, 1], BF16, name="gate_t")

            def emit_w_dma(p):
                def go():
                    nc.vector.tensor_copy(gate_t[:], wq_all[:, PW * p : PW * p + 1])
                    nc.sync.dma_start(out=wq_all[:, PW * p : PW * (p + 1)],
                                      in_=wq_d[:, PW * p : PW * (p + 1)])
                    nc.vector.tensor_copy(gate_t[:], wk_all[:, PW * p : PW * p + 1])
                    nc.sync.dma_start(out=wk_all[:, PW * p : PW * (p + 1)],
                                      in_=wk_d[:, PW * p : PW * (p + 1)])
                return go

            def emit_wp_dma():
                nc.vector.tensor_copy(gate_t[:], wp_all[:, 0:1])
                nc.sync.dma_start(out=wp_all[:], in_=wp_d)

            # ones-fill of the VA DoubleRow tiles (DVE, doesn't touch PE):
            # blocks (2p+k)*192 + 64:128 for all 8 (p,k) -> stride 192 x 8.
            for t in range(NKP):
                nc.vector.tensor_copy(
                    _ap3(VA[t][:, 64:65], 192, NP * 2, 1, 64), on_t[:]
                )

            # ---- QKV emission helpers ----
            def emit_qk_group(p, proj, ncq):
                """One 8-matmul psum group (+ DVE bias evac) for pair p.
                Returns list of closures emitting one instruction each."""
                w_all, bias_t, dst = (
                    (wq_all, bq_t, QT) if proj == 0 else (wk_all, bk_t, KT)
                )
                cs = slice(512 * ncq, 512 * (ncq + 1))
                state = {}

                def mk_mm(k):
                    def go():
                        if "pq" not in state:
                            state["pq"] = psqk.tile(
                                [128, 512], F32, tag="qk", name=f"pq_{p}_{proj}_{ncq}"
                            )
                        nc.tensor.matmul(
                            state["pq"][:],
                            w_all[:, PW * p + 128 * k : PW * p + 128 * (k + 1)],
                            xt_all[ncq][:, 512 * k : 512 * (k + 1)],
                            start=(k == 0), stop=(k == KT8 - 1), skip_group_check=True,
                        )

                    return go

                def evac():
                    nc.vector.tensor_scalar_add(
                        dst[p][:, cs], state["pq"][:], bias_t[:, p : p + 1]
                    )

                return [mk_mm(k) for k in range(KT8)] + [evac]

            def emit_v_group(nt):
                """V projection for key tile nt -> VA[nt//2] slot k=nt%2."""
                ncq, tt = divmod(nt, 4)
                t, k = divmod(nt, 2)
                state = {}

                def mk_mm(kk):
                    def go():
                        if "pv" not in state:
                            state["pv"] = psqk.tile(
                                [128, 512], F32, tag="qk", name=f"pv_{nt}"
                            )
                        nc.tensor.matmul(
                            state["pv"][:],
                            xt_all[ncq][:, 512 * kk + 128 * tt : 512 * kk + 128 * (tt + 1)],
                            wv_all[:, FC * kk : FC * (kk + 1)],
                            start=(kk == 0), stop=(kk == KT8 - 1), skip_group_check=True,
                        )

                    return go

                def evac():
                    va = VA[t]
                    pv = state["pv"]
                    base = 192 * k
                    nc.vector.tensor_copy(
                        _ap3(va[:, base : base + 1], 384, NP, 1, 64),
                        _cap(pv[:, 0:64], 128, NP, 64),
                    )
                    nc.vector.tensor_copy(
                        _ap3(va[:, base + 128 : base + 129], 384, NP, 1, 64),
                        _cap(pv[:, 64:128], 128, NP, 64),
                    )

                return [mk_mm(kk) for kk in range(KT8)] + [evac]

            # ---- output projection chunk emitter ----
            def emit_proj_group(c, ncq, fs=range(NP), partial_out=None, partial_in=None):
                """Y^T chunk: accumulating matmuls over pairs `fs`, then
                either stash the partial (partial_out) or add the stashed
                partial (partial_in) during the bf16 evac + DMA out."""
                cs = slice(512 * ncq, 512 * (ncq + 1))
                fs = list(fs)
                state = {}

                def mk_mm(f):
                    def go():
                        if "py" not in state:
                            state["py"] = psqk.tile(
                                [128, 512], F32, tag="qk", name=f"py_{c}_{ncq}_{fs[0]}"
                            )
                        nc.tensor.matmul(
                            state["py"][:],
                            wp_all[:, C * f + 128 * c : C * f + 128 * (c + 1)],
                            OT[f][:, cs],
                            start=(f == fs[0]), stop=(f == fs[-1]), skip_group_check=True,
                        )

                    return go

                def evac():
                    if partial_out is not None:
                        nc.vector.tensor_copy(partial_out[:], state["py"][:])
                        return
                    yb = pyb.tile([128, 512], BF16, tag="yb", name=f"yb_{c}_{ncq}")
                    if partial_in is not None:
                        nc.vector.tensor_add(yb[:], state["py"][:], partial_in[:])
                    else:
                        nc.vector.tensor_copy(yb[:], state["py"][:])
                    nc.sync.dma_start(out=yT_d[c, ncq, :, :], in_=yb[:])

                return [mk_mm(f) for f in fs] + [evac]

            # ---- deadline-scheduled filler groups ----
            # Each group's closures are spread EVENLY across steps
            # [avail, deadline] at build time. Even spreading (not
            # earliest-first) matters: draining filler early leaves the PE
            # micro-idling in later strips, which trips the HAM clock gate
            # back to 1.2 GHz and the whole pipeline slows ~1.5x.
            NSTEP = NP * NCQ * NMT
            emit_at = [[] for _ in range(NSTEP)]
            drain = []
            _sched_groups = []

            def add_group(avail, deadline, closures, psum=True):
                closures = list(closures)
                if avail >= NSTEP:
                    drain.extend(closures)
                    return
                _sched_groups.append(
                    {"avail": max(avail, 0), "dl": min(max(deadline, 0), NSTEP - 1),
                     "cl": closures, "psum": psum}
                )

            def assign_fillers():
                """Sequential tiler: groups in deadline order, closures packed
                at <=cap/step, bursting only when a deadline forces it. A
                psum group may not start before the second-previous psum
                group's last step — the filler PSUM pool has 2 buffers, and
                3+ concurrently open groups can deadlock the PE against the
                DVE evacuations."""
                load = [0] * NSTEP

                def cap(s):
                    # 12: DMA-bound crunch; 2 then 1: stretch the projection
                    # filler deep into pairs 2-3 (all-dummy stretches let the
                    # HAM clock gate re-throttle the PE into a sticky 1.2 GHz
                    # state); 3: output-projection region.
                    if s < NMT:
                        return 12
                    return 2 if s < 100 else (1 if s < 192 else 3)

                _sched_groups.sort(key=lambda g: (g["dl"], g["avail"]))
                open_last = [0, 0]
                for g in _sched_groups:
                    s = max(g["avail"], open_last[0] if g["psum"] else 0)
                    for c in g["cl"]:
                        while s < g["dl"] and load[s] >= cap(s):
                            s += 1
                        emit_at[s].append(c)
                        load[s] += 1
                    if g["psum"]:
                        open_last = [open_last[1], s]

            # V projections: VA pair t needed by PV at step 2t+1. avail is
            # aligned with the xT chunk DMA arrival so a stalled V matmul
            # doesn't head-of-line-block the in-order PE queue.
            for nt in range(NMT):
                av = 0 if nt < 4 else 4 * (nt // 4) - 1
                add_group(av, max(nt, 1), emit_v_group(nt))
            # pair-0 remaining Q/K chunks (chunk 0 is the prologue):
            for cq in range(1, NCQ):
                add_group(4 * cq - 3, 4 * cq - 2, emit_qk_group(0, 1, cq))  # K
                add_group(4 * cq - 1, 16 * cq - 2, emit_qk_group(0, 0, cq))  # Q
            # later pairs' Q/K (deadline 64p-2: the S at step 64p is emitted
            # during step 64p-1, so operands must be fully emitted before);
            # their weight DMAs are triggered from inside the loop first:
            add_group(14, 14, [emit_w_dma(1)], psum=False)
            add_group(24, 24, [emit_w_dma(2)], psum=False)
            add_group(56, 56, [emit_w_dma(3)], psum=False)
            add_group(80, 80, [emit_wp_dma], psum=False)
            for p in range(1, NP):
                base = 64 * p
                av = {1: 16, 2: 30, 3: 64}[p]
                for cq in range(NCQ):
                    add_group(av, base + 4 * cq - 2, emit_qk_group(p, 1, cq))
                    add_group(av, base + 16 * cq - 2, emit_qk_group(p, 0, cq))
            # output projection for qc: available after strip (p3, qc) ends;
            # staggered avail so the 8 c-groups don't all land on the same
            # steps, spread all the way to the end. The LAST chunk (qc3)
            # is split: pairs 0..2 accumulate into an SBUF partial during
            # pair-3's strips (their OT rows are long finished); only the
            # f=3 matmul + DVE add + DMA remain after the last step.
            for cq in range(NCQ - 1):
                av = 192 + 16 * cq + 16
                for c in range(C // 128):
                    add_group(av + c, NSTEP - 1, emit_proj_group(c, cq))
            yp = [
                sb.tile([128, 512], BF16, name=f"yp_{c}") for c in range(C // 128)
            ]
            for c in range(C // 128):
                add_group(
                    193 + 2 * c, 248,
                    emit_proj_group(c, 3, fs=range(NP - 1), partial_out=yp[c]),
                )
                drain.extend(
                    emit_proj_group(c, 3, fs=[NP - 1], partial_in=yp[c])
                )
            assign_fillers()

            # ---- prologue: pair-0 chunk-0 Q/K only ----
            for proj in range(2):
                for go in emit_qk_group(0, proj, 0):
                    go()

            # ---- attention strips ----
            # strip = (pair, 512-query-chunk). Both heads of the pair share
            # one [128, 1024] S psum tile: head A (rows 0:64 of K^T/Q^T) ->
            # cols 0:512, head B (rows 64:128) -> cols 512:1024; the two K=64
            # matmuls row-pack onto disjoint PE row-groups and run
            # concurrently. One exp covers both heads and writes fp8 into
            # half of the current E pair tile; PV fires every second step as
            # two DoubleRow matmuls over the key-tile pair (contraction 256).
            # Software-pipelined: S(i+1) is emitted before PV(i//2).
            steps = [(p, qc, mt) for p in range(NP) for qc in range(NCQ) for mt in range(NMT)]

            def emit_S(p, qc, mt):
                qs = slice(512 * qc, 512 * (qc + 1))
                ms = slice(128 * mt, 128 * (mt + 1))
                sa = psa.tile([128, 1024], F32, tag="sa", name=f"sa_{p}_{qc}_{mt}")
                nc.tensor.matmul(
                    sa[:, 0:512], KT[p][0:64, ms], QT[p][0:64, qs],
                    start=True, stop=True,
                )
                nc.tensor.matmul(
                    sa[:, 512:1024], KT[p][64:128, ms], QT[p][64:128, qs],
                    start=True, stop=True,
                )
                return sa

            ots = None
            et = None
            sa_next = emit_S(*steps[0])
            for i, (p, qc, mt) in enumerate(steps):
                if mt == 0:
                    ots = [
                        pso.tile([128, 512], F32, tag="o", name=f"o_{p}_{qc}_{j}")
                        for j in range(2)
                    ]
                if mt % 2 == 0:
                    et = pbe.tile([128, 2048], F8, tag="e", name=f"e_{p}_{qc}_{mt}")
                sa_cur = sa_next
                nc.scalar.activation(
                    et[:, 1024 * (mt % 2) : 1024 * (mt % 2) + 1024],
                    sa_cur[:], AF.Exp, scale=SCALE,
                )
                if i + 1 < len(steps):
                    sa_next = emit_S(*steps[i + 1])
                for go in emit_at[i]:
                    go()
                # pad the PE with zero-weight matmuls accumulating +0 into the
                # live O tile so per-step PE busy stays ~90%: micro-idle trips
                # the HAM clock gate into a sticky 1.2 GHz state. Skipped at
                # mt<2: there the O tile is freshly pool-rotated and a write
                # would stall the PE behind the previous strip's DVE
                # writeback.
                if mt >= 2:
                    pe_est = 321 + (482 if mt % 2 == 1 else 0) + 216 * len(emit_at[i])
                    d = 1000 - pe_est
                    ndum = 0 if d < 150 else (1 if d < 450 else (2 if d < 750 else 3))
                    for _ in range(ndum):
                        nc.tensor.matmul(
                            ots[1], z_t[:], QT[p][:, 0:512],
                            start=False, stop=False, skip_group_check=True,
                        )
                if mt % 2 == 1:
                    t = mt // 2
                    first, last = t == 0, t == NKP - 1
                    va = VA[t]
                    nc.tensor.matmul(
                        ots[0],
                        _ap3(va[:, 384 * p : 384 * p + 1], 192, 2, 1, 128),
                        _ap3(et[:, 0:1], 1024, 2, 1, 512),
                        start=first, stop=last, skip_group_check=True,
                        perf_mode=DR,
                    )
                    nc.tensor.matmul(
                        ots[1],
                        _ap3(va[:, 384 * p + 64 : 384 * p + 65], 192, 2, 1, 128),
                        _ap3(et[:, 512:513], 1024, 2, 1, 512),
                        start=first, stop=last, skip_group_check=True,
                        perf_mode=DR,
                    )
                if mt == NMT - 1:
                    qs = slice(512 * qc, 512 * (qc + 1))
                    for j in range(2):
                        o = ots[j]
                        # reciprocal_approx_fast mis-executes at base partition
                        # != 0: run it over the whole tile (unused rows produce
                        # garbage that is never read) and slice after.
                        rc = prc.tile([128, 512], F32, tag="rc", name=f"rc_{p}_{qc}_{j}")
                        nc.vector.reciprocal_approx_fast(rc[:], o[:])
                        osl, rcl = (
                            (o[0:64, :], rc[64:128, :]) if j == 0 else (o[64:128, :], rc[0:64, :])
                        )
                        nc.vector.tensor_mul(OT[p][64 * j : 64 * j + 64, qs], osl, rcl)
            # drain remaining fillers (final output projection chunks)
            for go in drain:
                go()

    nc.compile()
    return nc


def _get_nc():
    global _nc
    if _nc is None:
        try:
            import jax

            jax.config.update(
                "jax_compilation_cache_dir", os.path.expanduser("~/.cache/jax_bass")
            )
            jax.config.update("jax_persistent_cache_min_compile_time_secs", 0.0)
            jax.config.update("jax_persistent_cache_min_entry_size_bytes", 0)
        except Exception:
            pass
        _nc = _build()
    return _nc


def _wmerge(w, mdt):
    """(KT*128, F) -> [128, KT*F] partition-major merged layout."""
    kt = w.shape[0] // 128
    return np.ascontiguousarray(
        w.reshape(kt, 128, w.shape[1]).transpose(1, 0, 2).reshape(128, kt * w.shape[1]).astype(mdt)
    )


def _wmerge_pm(w, mdt):
    """(KT*128, NP*128) -> [128, NP*KT*128] pair-major merged layout:
    out[r, (p*KT + k)*128 + m] = w[128k + r, 128p + m]."""
    kt = w.shape[0] // 128
    npairs = w.shape[1] // 128
    return np.ascontiguousarray(
        w.reshape(kt, 128, npairs, 128)
        .transpose(1, 2, 0, 3)
        .reshape(128, npairs * kt * 128)
        .astype(mdt)
    )


def make_in_maps(inputs):
    import ml_dtypes

    mdt = ml_dtypes.bfloat16
    f8 = ml_dtypes.float8_e4m3fn
    x = np.asarray(inputs["x"], np.float32)
    Wq = np.asarray(inputs["Wq"], np.float32)
    Wk = np.asarray(inputs["Wk"], np.float32)
    Wv = np.asarray(inputs["Wv"], np.float32)
    Wp = np.asarray(inputs["Wp"], np.float32)
    bq = np.asarray(inputs["bq"], np.float32)
    bk = np.asarray(inputs["bk"], np.float32)
    ones = np.ones((128, 512), f8).view(np.uint8)
    in_maps = []
    for core in range(NCORES):
        b, g = core // 2, core % 2
        sl = slice(FC * g, FC * (g + 1))
        in_maps.append(
            {
                "xT": np.ascontiguousarray(
                    x[b].T.reshape(KT8, 128, NCQ, 512)
                    .transpose(2, 1, 0, 3)
                    .reshape(NCQ, 128, KT8 * 512)
                    .astype(mdt)
                ),
                "wq": _wmerge_pm(Wq[:, sl], mdt),
                "wk": _wmerge_pm(Wk[:, sl], mdt),
                "wv": _wmerge(Wv[:, sl], mdt),
                "wp": _wmerge(Wp[sl, :], mdt),
                "bq": np.ascontiguousarray(bq[sl].reshape(NP, 128).T),
                "bk": np.ascontiguousarray(bk[sl].reshape(NP, 128).T),
                "ones": ones,
            }
        )
    return in_maps


def assemble(results, inputs):
    Wp = np.asarray(inputs["Wp"], np.float32)
    bv = np.asarray(inputs["bv"], np.float32)
    bp = np.asarray(inputs["bp"], np.float32)
    fb = (bp.astype(np.float64) + bv.astype(np.float64) @ Wp.astype(np.float64)).astype(
        np.float32
    )
    out = np.empty((B, N, C), np.float32)
    for b in range(B):
        yt = (
            results[2 * b]["yT"].astype(np.float32)
            + results[2 * b + 1]["yT"].astype(np.float32)
        ).transpose(0, 2, 1, 3)
        out[b] = yt.reshape(C, N).T + fb
    return out


def run_on_device(inputs, trace=False, tmpdir=None):
    from concourse.bass_utils import run_bass_kernel_spmd

    nc = _get_nc()
    res = run_bass_kernel_spmd(
        nc, make_in_maps(inputs), list(range(NCORES)), trace=trace, tmpdir=tmpdir
    )
    return assemble(res.results, inputs), res


def kernel(**inputs):
    out, _ = run_on_device(inputs)
    return out


# revision 37
# speedup vs baseline: 1.0503x; 1.0076x over previous
"""Multi-head attention (B=4, N=2048, C=1024, H=16, D=64) on 8 trn2 cores.

Sharding: core c handles batch b = c//2 and head-group g = c%2 (8 heads,
512 channels). No collectives: the row-parallel output projection partials
are summed on host (2 cores per batch), with bp + bv@Wp folded in on host
(softmax rows sum to 1, so the v-bias contributes a constant per channel).

Device layout is transposed end-to-end (tokens on the free axis):
  xT [C, N] -> Q^T/K^T pair tiles [128, N] (rows 0:64 head 2p, 64:128 head
  2p+1) -> S^T tiles [keys, queries] via K=64 matmuls -> exp on ACT (no max
  subtraction; scores are O(1) by construction) -> exp output is fp8e4 and
  PV runs in fp8 DoubleRow over key-tile PAIRS (contraction 256): V_aug
  per pair tile [vA | ones64 | vB] x {k=0,1} interleaved; psum rows split
  into O_u and a 64-row replicated rowsum -> DVE reciprocal * mul -> O^T ->
  Y^T = Wp_g^T O^T (bf16).

Schedule: the kernel is ACT(exp)-bound (256 exps x ~1.07us = 274us); the
whole schedule aims to keep ACT 100% busy from ~10us on. Prologue is only
wq+x0+wk DMA and the pair-0 chunk-0 Q/K projection; all other projections
(V for all 16 key tiles, remaining Q/K chunks, later pairs, output
projection) are deadline-scheduled as PE filler inside the attention
strips. A dummy-matmul burst at t0 (on the resident ones tile) keeps the
PE HAM clock gate warm through the initial DMA wait so the first real
matmuls run at 2.4 GHz.

Matmul operand dtypes: bf16 for projections and S (accuracy-critical),
fp8e4m3 for the PV DoubleRow pair (E = exp output, V_aug); accumulation
is always fp32 in PSUM. Measured fro rel err ~1.4e-2 (gate 2e-2).
"""

import os
import sys

sys.path.insert(0, "/opt/trn_rl_repo")

import numpy as np

B, N, C, H = 4, 2048, 1024, 16
D = C // H
SCALE = D**-0.5
NCORES = 8
FC = 512  # channels per core
NP = 4  # head pairs per core
KT8 = C // 128  # contraction tiles
NCQ = N // 512  # n-chunks of 512
NMT = N // 128  # key tiles
NKP = NMT // 2  # key-tile pairs (DoubleRow)

_nc = None


def _cap(ap_slice, block_step, nblocks, width):
    """2-free-dim AP: nblocks blocks of `width` cols, stride block_step."""
    import concourse.bass as bass

    lst = [list(p) for p in ap_slice.ap]
    assert len(lst) == 2 and lst[1][0] == 1, lst
    return bass.AP(
        ap_slice.tensor, ap_slice.offset, [lst[0], [block_step, nblocks], [1, width]]
    )


def _ap3(ap_slice, d1_step, d1_n, d2_step, d2_n):
    """3-free-dim AP for DoubleRow operands: [part, [d1_step,d1_n], [d2_step,d2_n]]."""
    import concourse.bass as bass

    lst = [list(p) for p in ap_slice.ap]
    return bass.AP(
        ap_slice.tensor, ap_slice.offset, [lst[0], [d1_step, d1_n], [d2_step, d2_n]]
    )


def _build():
    import concourse.bacc as bacc
    import concourse.mybir as mybir
    import concourse.tile as tile

    F32 = mybir.dt.float32
    BF16 = mybir.dt.bfloat16
    F8 = mybir.dt.float8e4
    AF = mybir.ActivationFunctionType
    DR = mybir.MatmulPerfMode.DoubleRow

    nc = bacc.Bacc("TRN2", target_bir_lowering=False, debug=False, num_devices=NCORES)

    xT_d = nc.dram_tensor("xT", (NCQ, 128, KT8 * 512), BF16, kind="ExternalInput").ap()
    # wq/wk are pair-major [128, NP, KT8, 128] so pair 0's slices can DMA
    # first; wv keeps the k-major merged layout (its matmuls read full rows).
    wq_d = nc.dram_tensor("wq", (128, KT8 * FC), BF16, kind="ExternalInput").ap()
    wk_d = nc.dram_tensor("wk", (128, KT8 * FC), BF16, kind="ExternalInput").ap()
    wv_d = nc.dram_tensor("wv", (128, KT8 * FC), BF16, kind="ExternalInput").ap()
    wp_d = nc.dram_tensor("wp", (128, NP * C), BF16, kind="ExternalInput").ap()
    bq_d = nc.dram_tensor("bq", (128, NP), F32, kind="ExternalInput").ap()
    bk_d = nc.dram_tensor("bk", (128, NP), F32, kind="ExternalInput").ap()
    on_d = nc.dram_tensor("ones", (128, 512), F8, kind="ExternalInput").ap()
    yT_d = nc.dram_tensor("yT", (C // 128, NCQ, 128, 512), BF16, kind="ExternalOutput").ap()

    with tile.TileContext(nc) as tc:
        with (
            tc.tile_pool(name="sb", bufs=1) as sb,
            tc.tile_pool(name="pe_", bufs=3) as pbe,
            tc.tile_pool(name="prc", bufs=4) as prc,
            tc.tile_pool(name="pyb", bufs=4) as pyb,
            tc.tile_pool(name="psa", bufs=2, space="PSUM") as psa,
            tc.tile_pool(name="psqk", bufs=2, space="PSUM") as psqk,
            tc.tile_pool(name="pso", bufs=2, space="PSUM") as pso,
        ):
            # ---- resident tiles + DMAs ----
            QT = [sb.tile([128, N], BF16, name=f"qt{p}") for p in range(NP)]
            KT = [sb.tile([128, N], BF16, name=f"kt{p}") for p in range(NP)]
            # V_aug DoubleRow tiles: per key-tile pair t, layout
            # [128 keys, (2p+k)*192 + {vA 0:64 | ones 64:128 | vB 128:192}]
            VA = [sb.tile([128, 192 * NP * 2], F8, name=f"va{t}") for t in range(NKP)]
            OT = [sb.tile([128, N], BF16, name=f"ot{p}") for p in range(NP)]
            bq_t = sb.tile([128, NP], F32, name="bq_t")
            bk_t = sb.tile([128, NP], F32, name="bk_t")
            on_t = sb.tile([128, 512], F8, name="on_t")
            z_t = sb.tile([128, 128], BF16, name="z_t")
            nc.sync.dma_start(out=bq_t[:], in_=bq_d)
            nc.sync.dma_start(out=bk_t[:], in_=bk_d)
            nc.sync.dma_start(out=on_t[:], in_=on_d)
            nc.vector.memset(z_t[:], 0.0)

            # ---- HAM warm-up at t0: dummy matmuls on the ones tile keep the
            # PE activity monitor busy through the wq/x0 DMA wait so the
            # first projection matmuls run at 2.4 GHz.
            warm = psqk.tile([128, 512], F32, tag="qk", name="warm")
            for _ in range(16):
                nc.tensor.matmul(
                    warm[:], on_t[:, 0:128], on_t[:],
                    start=True, stop=True, skip_group_check=True,
                )

            # DMA priority: wq/wk pair-0 slices + xT chunk 0 first (gates the
            # first exp), wv next (V filler for the first strip), then the
            # remaining pairs' wq/wk, remaining xT, wp last.
            PW = KT8 * 128  # per-pair width in the pair-major wq/wk layout
            wq_all = sb.tile([128, KT8 * FC], BF16, name="wq_all")
            wk_all = sb.tile([128, KT8 * FC], BF16, name="wk_all")
            wv_all = sb.tile([128, KT8 * FC], BF16, name="wv_all")
            # The DMA path delivers only ~120 GB/s regardless of issue
            # pattern (latency-bound SDMA round-trips), so the x chunks are
            # simply ordered by first use; each is split in column halves so
            # low-k matmuls can start before the full chunk lands.
            HW2 = KT8 * 256
            nc.sync.dma_start(out=wq_all[:, 0:PW], in_=wq_d[:, 0:PW])
            nc.sync.dma_start(out=wk_all[:, 0:PW], in_=wk_d[:, 0:PW])

            def _xall(ncq):
                t = sb.tile([128, KT8 * 512], BF16, name=f"xt_{ncq}")
                nc.sync.dma_start(out=t[:, 0:HW2], in_=xT_d[ncq, :, 0:HW2])
                nc.sync.dma_start(out=t[:, HW2:], in_=xT_d[ncq, :, HW2:])
                return t

            xt_all = [_xall(0), _xall(1)]
            nc.sync.dma_start(out=wv_all[:, 0:HW2], in_=wv_d[:, 0:HW2])
            nc.sync.dma_start(out=wv_all[:, HW2:], in_=wv_d[:, HW2:])
            xt_all.append(_xall(2))
            xt_all.append(_xall(3))
            # Later pairs' weights and wp are DMA'd from inside the step
            # loop (the HWDGE queues interleave all pending transfers, so
            # issuing everything at t0 starves the chunks the first strip
            # is already waiting on). A dummy DVE read of the destination
            # creates a WAR hazard that holds the trigger back to the wall
            # time of the step it is emitted at.
            wp_all = sb.tile([128, NP * C], BF16, name="wp_all")
            gate_t = sb.tile([128, 1], BF16, name="gate_t")

            def emit_w_dma(p):
                def go():
                    nc.vector.tensor_copy(gate_t[:], wq_all[:, PW * p : PW * p + 1])
                    nc.sync.dma_start(out=wq_all[:, PW * p : PW * (p + 1)],
                                      in_=wq_d[:, PW * p : PW * (p + 1)])
                    nc.vector.tensor_copy(gate_t[:], wk_all[:, PW * p : PW * p + 1])
                    nc.sync.dma_start(out=wk_all[:, PW * p : PW * (p + 1)],
                                      in_=wk_d[:, PW * p : PW * (p + 1)])
                return go

            def emit_wp_dma():
                nc.vector.tensor_copy(gate_t[:], wp_all[:, 0:1])
                nc.sync.dma_start(out=wp_all[:], in_=wp_d)

            # ones-fill of the VA DoubleRow tiles (DVE, doesn't touch PE):
            # blocks (2p+k)*192 + 64:128 for all 8 (p,k) -> stride 192 x 8.
            for t in range(NKP):
                nc.vector.tensor_copy(
                    _ap3(VA[t][:, 64:65], 192, NP * 2, 1, 64), on_t[:]
                )

            # ---- QKV emission helpers ----
            def emit_qk_group(p, proj, ncq):
                """One 8-matmul psum group (+ DVE bias evac) for pair p.
                Returns list of closures emitting one instruction each."""
                w_all, bias_t, dst = (
                    (wq_all, bq_t, QT) if proj == 0 else (wk_all, bk_t, KT)
                )
                cs = slice(512 * ncq, 512 * (ncq + 1))
                state = {}

                def mk_mm(k):
                    def go():
                        if "pq" not in state:
                            state["pq"] = psqk.tile(
                                [128, 512], F32, tag="qk", name=f"pq_{p}_{proj}_{ncq}"
                            )
                        nc.tensor.matmul(
                            state["pq"][:],
                            w_all[:, PW * p + 128 * k : PW * p + 128 * (k + 1)],
                            xt_all[ncq][:, 512 * k : 512 * (k + 1)],
                            start=(k == 0), stop=(k == KT8 - 1), skip_group_check=True,
                        )

                    return go

                def evac():
                    nc.vector.tensor_scalar_add(
                        dst[p][:, cs], state["pq"][:], bias_t[:, p : p + 1]
                    )

                return [mk_mm(k) for k in range(KT8)] + [evac]

            def emit_v_group(nt):
                """V projection for key tile nt -> VA[nt//2] slot k=nt%2."""
                ncq, tt = divmod(nt, 4)
                t, k = divmod(nt, 2)
                state = {}

                def mk_mm(kk):
                    def go():
                        if "pv" not in state:
                            state["pv"] = psqk.tile(
                                [128, 512], F32, tag="qk", name=f"pv_{nt}"
                            )
                        nc.tensor.matmul(
                            state["pv"][:],
                            xt_all[ncq][:, 512 * kk + 128 * tt : 512 * kk + 128 * (tt + 1)],
                            wv_all[:, FC * kk : FC * (kk + 1)],
                            start=(kk == 0), stop=(kk == KT8 - 1), skip_group_check=True,
                        )

                    return go

                def evac():
                    va = VA[t]
                    pv = state["pv"]
                    base = 192 * k
                    nc.vector.tensor_copy(
                        _ap3(va[:, base : base + 1], 384, NP, 1, 64),
                        _cap(pv[:, 0:64], 128, NP, 64),
                    )
                    nc.vector.tensor_copy(
                        _ap3(va[:, base + 128 : base + 129], 384, NP, 1, 64),
                        _cap(pv[:, 64:128], 128, NP, 64),
                    )

                return [mk_mm(kk) for kk in range(KT8)] + [evac]

            # ---- output projection chunk emitter ----
            def emit_proj_group(c, ncq, fs=range(NP), partial_out=None, partial_in=None):
                """Y^T chunk: accumulating matmuls over pairs `fs`, then
                either stash the partial (partial_out) or add the stashed
                partial (partial_in) during the bf16 evac + DMA out."""
                cs = slice(512 * ncq, 512 * (ncq + 1))
                fs = list(fs)
                state = {}

                def mk_mm(f):
                    def go():
                        if "py" not in state:
                            state["py"] = psqk.tile(
                                [128, 512], F32, tag="qk", name=f"py_{c}_{ncq}_{fs[0]}"
                            )
                        nc.tensor.matmul(
                            state["py"][:],
                            wp_all[:, C * f + 128 * c : C * f + 128 * (c + 1)],
                            OT[f][:, cs],
                            start=(f == fs[0]), stop=(f == fs[-1]), skip_group_check=True,
                        )

                    return go

                def evac():
                    if partial_out is not None:
                        nc.vector.tensor_copy(partial_out[:], state["py"][:])
                        return
                    yb = pyb.tile([128, 512], BF16, tag="yb", name=f"yb_{c}_{ncq}")
                    if partial_in is not None:
                        nc.vector.tensor_add(yb[:], state["py"][:], partial_in[:])
                    else:
                        nc.vector.tensor_copy(yb[:], state["py"][:])
                    nc.sync.dma_start(out=yT_d[c, ncq, :, :], in_=yb[:])

                return [mk_mm(f) for f in fs] + [evac]

            # ---- deadline-scheduled filler groups ----
            # Each group's closures are spread EVENLY across steps
            # [avail, deadline] at build time. Even spreading (not
            # earliest-first) matters: draining filler early leaves the PE
            # micro-idling in later strips, which trips the HAM clock gate
            # back to 1.2 GHz and the whole pipeline slows ~1.5x.
            NSTEP = NP * NCQ * NMT
            emit_at = [[] for _ in range(NSTEP)]
            drain = []
            _sched_groups = []

            def add_group(avail, deadline, closures, psum=True):
                closures = list(closures)
                if avail >= NSTEP:
                    drain.extend(closures)
                    return
                _sched_groups.append(
                    {"avail": max(avail, 0), "dl": min(max(deadline, 0), NSTEP - 1),
                     "cl": closures, "psum": psum}
                )

            def assign_fillers():
                """Sequential tiler: groups in deadline order, closures packed
                at <=cap/step, bursting only when a deadline forces it. A
                psum group may not start before the second-previous psum
                group's last step — the filler PSUM pool has 2 buffers, and
                3+ concurrently open groups can deadlock the PE against the
                DVE evacuations."""
                load = [0] * NSTEP

                def cap(s):
                    # 12: DMA-bound crunch; 2 then 1: stretch the projection
                    # filler deep into pairs 2-3 (all-dummy stretches let the
                    # HAM clock gate re-throttle the PE into a sticky 1.2 GHz
                    # state); 3: output-projection region.
                    if s < NMT:
                        return 12
                    return 2 if s < 100 else (1 if s < 192 else 3)

                _sched_groups.sort(key=lambda g: (g["dl"], g["avail"]))
                open_last = [0, 0]
                for g in _sched_groups:
                    s = max(g["avail"], open_last[0] if g["psum"] else 0)
                    for c in g["cl"]:
                        while s < g["dl"] and load[s] >= cap(s):
                            s += 1
                        emit_at[s].append(c)
                        load[s] += 1
                    if g["psum"]:
                        open_last = [open_last[1], s]

            # V projections: VA pair t needed by PV at step 2t+1. avail is
            # aligned with the xT chunk DMA arrival so a stalled V matmul
            # doesn't head-of-line-block the in-order PE queue.
            for nt in range(NMT):
                av = 0 if nt < 4 else 4 * (nt // 4) - 1
                add_group(av, max(nt, 1), emit_v_group(nt))
            # pair-0 remaining Q/K chunks (chunk 0 is the prologue):
            for cq in range(1, NCQ):
                add_group(4 * cq - 3, 4 * cq - 2, emit_qk_group(0, 1, cq))  # K
                add_group(4 * cq - 1, 16 * cq - 2, emit_qk_group(0, 0, cq))  # Q
            # later pairs' Q/K (deadline 64p-2: the S at step 64p is emitted
            # during step 64p-1, so operands must be fully emitted before);
            # their weight DMAs are triggered from inside the loop first:
            add_group(14, 14, [emit_w_dma(1)], psum=False)
            add_group(24, 24, [emit_w_dma(2)], psum=False)
            add_group(56, 56, [emit_w_dma(3)], psum=False)
            add_group(80, 80, [emit_wp_dma], psum=False)
            for p in range(1, NP):
                base = 64 * p
                av = {1: 16, 2: 30, 3: 64}[p]
                for cq in range(NCQ):
                    add_group(av, base + 4 * cq - 2, emit_qk_group(p, 1, cq))
                    add_group(av, base + 16 * cq - 2, emit_qk_group(p, 0, cq))
            # output projection for qc: available after strip (p3, qc) ends;
            # staggered avail so the 8 c-groups don't all land on the same
            # steps, spread all the way to the end. The LAST chunk (qc3)
            # is split: pairs 0..2 accumulate into an SBUF partial during
            # pair-3's strips (their OT rows are long finished); only the
            # f=3 matmul + DVE add + DMA remain after the last step.
            for cq in range(NCQ - 1):
                av = 192 + 16 * cq + 16
                for c in range(C // 128):
                    add_group(av + c, NSTEP - 1, emit_proj_group(c, cq))
            yp = [
                sb.tile([128, 512], BF16, name=f"yp_{c}") for c in range(C // 128)
            ]
            for c in range(C // 128):
                add_group(
                    193 + 2 * c, 248,
                    emit_proj_group(c, 3, fs=range(NP - 1), partial_out=yp[c]),
                )
                drain.extend(
                    emit_proj_group(c, 3, fs=[NP - 1], partial_in=yp[c])
                )
            assign_fillers()

            # ---- prologue: pair-0 chunk-0 Q/K only ----
            for proj in range(2):
                for go in emit_qk_group(0, proj, 0):
                    go()

            # ---- attention strips ----
            # strip = (pair, 512-query-chunk). Both heads of the pair share
            # one [128, 1024] S psum tile: head A (rows 0:64 of K^T/Q^T) ->
            # cols 0:512, head B (rows 64:128) -> cols 512:1024; the two K=64
            # matmuls row-pack onto disjoint PE row-groups and run
            # concurrently. One exp covers both heads and writes fp8 into
            # half of the current E pair tile; PV fires every second step as
            # two DoubleRow matmuls over the key-tile pair (contraction 256).
            # Software-pipelined: S(i+1) is emitted before PV(i//2).
            steps = [(p, qc, mt) for p in range(NP) for qc in range(NCQ) for mt in range(NMT)]

            def emit_S(p, qc, mt):
                qs = slice(512 * qc, 512 * (qc + 1))
                ms = slice(128 * mt, 128 * (mt + 1))
                sa = psa.tile([128, 1024], F32, tag="sa", name=f"sa_{p}_{qc}_{mt}")
                nc.tensor.matmul(
                    sa[:, 0:512], KT[p][0:64, ms], QT[p][0:64, qs],
                    start=True, stop=True,
                )
                nc.tensor.matmul(
                    sa[:, 512:1024], KT[p][64:128, ms], QT[p][64:128, qs],
                    start=True, stop=True,
                )
                return sa

            ots = None
            et = None
            sa_next = emit_S(*steps[0])
            for i, (p, qc, mt) in enumerate(steps):
                if mt == 0:
                    ots = [
                        pso.tile([128, 512], F32, tag="o", name=f"o_{p}_{qc}_{j}")
                        for j in range(2)
                    ]
                if mt % 2 == 0:
                    et = pbe.tile([128, 2048], F8, tag="e", name=f"e_{p}_{qc}_{mt}")
                sa_cur = sa_next
                nc.scalar.activation(
                    et[:, 1024 * (mt % 2) : 1024 * (mt % 2) + 1024],
                    sa_cur[:], AF.Exp, scale=SCALE,
                )
                if i + 1 < len(steps):
                    sa_next = emit_S(*steps[i + 1])
                for go in emit_at[i]:
                    go()
                # pad the PE with zero-weight matmuls accumulating +0 into the
                # live O tile so per-step PE busy stays ~90%: micro-idle trips
                # the HAM clock gate into a sticky 1.2 GHz state. Skipped at
                # mt<2: there the O tile is freshly pool-rotated and a write
                # would stall the PE behind the previous strip's DVE
                # writeback.
                if mt >= 2:
                    pe_est = 321 + (482 if mt % 2 == 1 else 0) + 216 * len(emit_at[i])
                    d = 1000 - pe_est
                    ndum = 0 if d < 150 else (1 if d < 450 else (2 if d < 750 else 3))
                    for _ in range(ndum):
                        nc.tensor.matmul(
                            ots[1], z_t[:], QT[p][:, 0:512],
                            start=False, stop=False, skip_group_check=True,
                        )
                if mt % 2 == 1:
                    t = mt // 2
                    first, last = t == 0, t == NKP - 1
                    va = VA[t]
                    nc.tensor.matmul(
                        ots[0],
                        _ap3(va[:, 384 * p : 384 * p + 1], 192, 2, 1, 128),
                        _ap3(et[:, 0:1], 1024, 2, 1, 512),
                        start=first, stop=last, skip_group_check=True,
                        perf_mode=DR,
                    )
                    nc.tensor.matmul(
                        ots[1],
                        _ap3(va[:, 384 * p + 64 : 384 * p + 65], 192, 2, 1, 128),
                        _ap3(et[:, 512:513], 1024, 2, 1, 512),
                        start=first, stop=last, skip_group_check=True,
                        perf_mode=DR,
                    )
                if mt == NMT - 1:
                    qs = slice(512 * qc, 512 * (qc + 1))
                    for j in range(2):
                        o = ots[j]
                        # reciprocal_approx_fast mis-executes at base partition
                        # != 0: run it over the whole tile (unused rows produce
                        # garbage that is never read) and slice after.
                        rc = prc.tile([128, 512], F32, tag="rc", name=f"rc_{p}_{qc}_{j}")
                        nc.vector.reciprocal_approx_fast(rc[:], o[:])
                        osl, rcl = (
                            (o[0:64, :], rc[64:128, :]) if j == 0 else (o[64:128, :], rc[0:64, :])
                        )
                        nc.vector.tensor_mul(OT[p][64 * j : 64 * j + 64, qs], osl, rcl)
            # drain remaining fillers (final output projection chunks)
            for go in drain:
                go()

    nc.compile()
    return nc


def _get_nc():
    global _nc
    if _nc is None:
        try:
            import jax

            jax.config.update(
                "jax_compilation_cache_dir", os.path.expanduser("~/.cache/jax_bass")
            )
            jax.config.update("jax_persistent_cache_min_compile_time_secs", 0.0)
            jax.config.update("jax_persistent_cache_min_entry_size_bytes", 0)
        except Exception:
            pass
        _nc = _build()
    return _nc


def _wmerge(w, mdt):
    """(KT*128, F) -> [128, KT*F] partition-major merged layout."""
    kt = w.shape[0] // 128
    return np.ascontiguousarray(
        w.reshape(kt, 128, w.shape[1]).transpose(1, 0, 2).reshape(128, kt * w.shape[1]).astype(mdt)
    )


def _wmerge_pm(w, mdt):
    """(KT*128, NP*128) -> [128, NP*KT*128] pair-major merged layout:
    out[r, (p*KT + k)*128 + m] = w[128k + r, 128p + m]."""
    kt = w.shape[0] // 128
    npairs = w.shape[1] // 128
    return np.ascontiguousarray(
        w.reshape(kt, 128, npairs, 128)
        .transpose(1, 2, 0, 3)
        .reshape(128, npairs * kt * 128)
        .astype(mdt)
    )


def make_in_maps(inputs):
    import ml_dtypes

    mdt = ml_dtypes.bfloat16
    f8 = ml_dtypes.float8_e4m3fn
    x = np.asarray(inputs["x"], np.float32)
    Wq = np.asarray(inputs["Wq"], np.float32)
    Wk = np.asarray(inputs["Wk"], np.float32)
    Wv = np.asarray(inputs["Wv"], np.float32)
    Wp = np.asarray(inputs["Wp"], np.float32)
    bq = np.asarray(inputs["bq"], np.float32)
    bk = np.asarray(inputs["bk"], np.float32)
    ones = np.ones((128, 512), f8).view(np.uint8)
    in_maps = []
    for core in range(NCORES):
        b, g = core // 2, core % 2
        sl = slice(FC * g, FC * (g + 1))
        in_maps.append(
            {
                "xT": np.ascontiguousarray(
                    x[b].T.reshape(KT8, 128, NCQ, 512)
                    .transpose(2, 1, 0, 3)
                    .reshape(NCQ, 128, KT8 * 512)
                    .astype(mdt)
                ),
                "wq": _wmerge_pm(Wq[:, sl], mdt),
                "wk": _wmerge_pm(Wk[:, sl], mdt),
                "wv": _wmerge(Wv[:, sl], mdt),
                "wp": _wmerge(Wp[sl, :], mdt),
                "bq": np.ascontiguousarray(bq[sl].reshape(NP, 128).T),
                "bk": np.ascontiguousarray(bk[sl].reshape(NP, 128).T),
                "ones": ones,
            }
        )
    return in_maps


def assemble(results, inputs):
    Wp = np.asarray(inputs["Wp"], np.float32)
    bv = np.asarray(inputs["bv"], np.float32)
    bp = np.asarray(inputs["bp"], np.float32)
    fb = (bp.astype(np.float64) + bv.astype(np.float64) @ Wp.astype(np.float64)).astype(
        np.float32
    )
    out = np.empty((B, N, C), np.float32)
    for b in range(B):
        yt = (
            results[2 * b]["yT"].astype(np.float32)
            + results[2 * b + 1]["yT"].astype(np.float32)
        ).transpose(0, 2, 1, 3)
        out[b] = yt.reshape(C, N).T + fb
    return out


def run_on_device(inputs, trace=False, tmpdir=None):
    from concourse.bass_utils import run_bass_kernel_spmd

    nc = _get_nc()
    res = run_bass_kernel_spmd(
        nc, make_in_maps(inputs), list(range(NCORES)), trace=trace, tmpdir=tmpdir
    )
    return assemble(res.results, inputs), res


def kernel(**inputs):
    out, _ = run_on_device(inputs)
    return out
